# revision 39
# baseline (speedup 1.0000x reference)
"""DeepseekV2 MLA attention prefill kernel for 8 Trainium2 NeuronCores.

Sharding: 2-way data-parallel over batch x 4-way tensor-parallel over heads
(4 heads per core).  The raw q down-projection (+ rstd of its RMS norm) is
computed on an S/4 slice per core and exchanged with one in-group AllGather;
the RMS normalization is folded into the q up-projection output after the
gather.  The compressed-KV path is replicated at full S on every core and
computed while the gather is in flight.  Per-head up-projections, attention
and the output projection are computed locally; o_proj partial sums are
reduced on the host during unsharding.

Key scheduling/efficiency points (v4):
 - score matmuls run in fp8e4 DoubleRow: the two 128-deep k-subtiles are
   [k_nope | (k_pe ; zeros)], so one PE pass per 128x512 score block covers
   the full 192-dim contraction (rope included); q/k packs are built by the
   DVE/DMA on the side.  Everything else stays bf16 (fp8 there fails the
   2e-2 tolerance; scores measured 1.0e-2 in emulation).
 - the collective lives alone on the gpsimd queue; RMS rstd broadcasts are
   done by a K=1 PE matmul against a ones row so the kv-norm never blocks
   behind the 90us gather.
 - DMA priority at startup: only the q-down critical stream (xt_loc + wdq)
   is issued first; all other weights follow it on the same queue.
 - exp is evaluated over [128, 1024] pairs of score banks (halves ScalarE
   instruction overhead); attention context is evacuated raw and the
   softmax 1/sum is applied during a later DVE pass, so the single-bank
   ctx accumulator frees immediately at head boundaries.
 - PV and row-sum matmuls restrict their free dim on diagonal blocks.

Layouts: activations are feature-major ([D, S]); scores are computed
transposed ([s_k, s_q]) so PV needs no transposes.  RoPE uses host-side
permuted/sign-folded weight columns.  PSUM accumulation fp32 throughout.
"""
import sys
sys.path.insert(0, "/opt/trn_rl_repo")

import math
import numpy as np
import ml_dtypes

import concourse.bass as bass
import concourse.tile as tile
from concourse import bacc, mybir
from concourse.bass_utils import run_bass_kernel_spmd

# ---- problem constants (hardcoded; kernel.py must be self-contained) ----
B, S, HID, H = 2, 2048, 2048, 16
Q_LORA, KV_LORA = 1536, 512
D_NOPE, D_ROPE, D_V = 128, 64, 128
D_Q = D_NOPE + D_ROPE
EPS = 1e-6
ROPE_THETA = 10000.0
N_CORES = 8
HPC = 4                      # heads per core
GROUPS = [[0, 1, 2, 3], [4, 5, 6, 7]]

KC = HID // 128              # 16
QC = Q_LORA // 128           # 12
VC = KV_LORA // 128          # 4
NSK = S // 128               # 16 key blocks

F32 = mybir.dt.float32
BF16 = mybir.dt.bfloat16
F8 = mybir.dt.float8e4
MM_DT = BF16
DR = mybir.MatmulPerfMode.DoubleRow

SCALE = 1.0 / math.sqrt(D_Q)

_CACHE = {}


# ---------------------------------------------------------------- builder --
def build_kernel(mm_dt=MM_DT):
    s_loc = S // 4

    nc = bacc.Bacc("TRN2", target_bir_lowering=False, debug=False,
                   num_devices=N_CORES)

    xt = nc.dram_tensor("xt", [HID, S], mm_dt, kind="ExternalInput")
    xt_loc = nc.dram_tensor("xt_loc", [HID, s_loc], mm_dt, kind="ExternalInput")
    wdq = nc.dram_tensor("wdq", [HID, Q_LORA], mm_dt, kind="ExternalInput")
    wuq = nc.dram_tensor("wuq", [128, QC, HPC * 256], mm_dt, kind="ExternalInput")
    wkva = nc.dram_tensor("wkva", [128, KC, KV_LORA + 2 * D_ROPE], mm_dt,
                          kind="ExternalInput")
    wkvb = nc.dram_tensor("wkvb", [128, VC, HPC, 256], mm_dt, kind="ExternalInput")
    ow = nc.dram_tensor("ow", [D_V, HPC, HID], mm_dt, kind="ExternalInput")
    cos_f = nc.dram_tensor("cos_f", [D_ROPE, S], mm_dt, kind="ExternalInput")
    sin_f = nc.dram_tensor("sin_f", [D_ROPE, S], mm_dt, kind="ExternalInput")
    masks = nc.dram_tensor("masks", [128, 4, 512], F8, kind="ExternalInput")
    out_t = nc.dram_tensor("out_t", [HID, S], mm_dt, kind="ExternalOutput")

    with tile.TileContext(nc) as tc:
        import contextlib
        ctx = contextlib.ExitStack()
        with ctx:
            persist = ctx.enter_context(tc.tile_pool(name="persist", bufs=1))
            wpool = ctx.enter_context(tc.tile_pool(name="wpool", bufs=3))
            spool = ctx.enter_context(tc.tile_pool(name="spool", bufs=2))
            xpool = ctx.enter_context(tc.tile_pool(name="xpool", bufs=2))
            # PSUM: ppool 2 + pscore 2x2banks + pctx 1 + psums 1 = 8 banks
            ppool = ctx.enter_context(tc.tile_pool(name="ppool", bufs=2, space="PSUM"))
            pscore = ctx.enter_context(tc.tile_pool(name="pscore", bufs=2, space="PSUM"))
            pctx = ctx.enter_context(tc.tile_pool(name="pctx", bufs=1, space="PSUM"))
            psums = ctx.enter_context(tc.tile_pool(name="psums", bufs=1, space="PSUM"))
            dram = ctx.enter_context(tc.tile_pool(name="dram", bufs=1, space="DRAM"))

            ones_sb = persist.tile([128, 1], mm_dt, tag="ones")
            nc.vector.memset(ones_sb, 1.0)
            onesr_sb = persist.tile([1, 128], mm_dt, tag="onesr")
            nc.vector.memset(onesr_sb, 1.0)
            eps_sb = persist.tile([1, 1], F32, tag="eps")
            nc.vector.memset(eps_sb, EPS)

            # fused fp8 key pack: [d(128), h, skt, {nope | rope}, s_k(128)];
            # rope rows 64-127 are zero so the q-side values there are inert
            kf_sb = persist.tile([128, HPC, NSK, 2, 128], F8, tag="kf")
            nc.vector.memset(kf_sb[64:128, :, :, 1, :], 0.0)
            # fp8 q packs: rows 64-127 of the rope subtile are never written,
            # and uninitialized fp8 bytes can decode as NaN (NaN*0=NaN in the
            # PE), so zero them once up front.
            qf_t = {}
            for h in range(HPC):
                qf_t[h] = persist.tile([128, 2, 512], F8, tag="qf_h%d" % h,
                                       name="qf%d" % h)
                nc.vector.memset(qf_t[h][64:128, 1, :], 0.0)

            # ---- q-down critical DMA streams: xt_loc on the scalar ring,
            # wdq alone on the sync ring (two HWDGE rings run in parallel).
            # xt chunks 0-1 are prestaged on the scalar ring as well, so
            # kv_a's first half needs no DMA while the gather throttles
            # regular transfers to ~40 GB/s; chunks 2-3 stream as before
            # (4.2MB fits in what the gather window can still deliver).
            xtl_sb = persist.tile([128, KC, 512], mm_dt, tag="xtl")
            xt2_sb = persist.tile([128, KC, 2, 512], mm_dt, tag="xt2")
            mgs = [list(range(g, g + 4)) for g in range(0, QC, 4)]
            wdq_t = {}
            for k in range(KC):
                nc.scalar.dma_start(out=xtl_sb[:, k, :],
                                    in_=xt_loc.ap()[k * 128:(k + 1) * 128, :])
                wt = wpool.tile([128, 512], mm_dt, tag="w_s1", bufs=6)
                nc.sync.dma_start(
                    out=wt, in_=wdq.ap()[k * 128:(k + 1) * 128, 0:512])
                wdq_t[(0, k)] = wt
            for c in range(2):
                for k in range(KC):
                    nc.scalar.dma_start(
                        out=xt2_sb[:, k, c, :],
                        in_=xt.ap()[k * 128:(k + 1) * 128,
                                    c * 512:(c + 1) * 512])
            for gi in range(1, 3):
                for k in range(KC):
                    wt = wpool.tile([128, 512], mm_dt, tag="w_s1", bufs=6)
                    nc.sync.dma_start(
                        out=wt,
                        in_=wdq.ap()[k * 128:(k + 1) * 128,
                                     gi * 512:(gi + 1) * 512])
                    wdq_t[(gi, k)] = wt

            # gather buffers (DRAM): 12 raw q-down chunks + broadcast rstd
            GR = Q_LORA + 128
            g_in = dram.tile([GR, s_loc], mm_dt)
            g_out = dram.tile([4 * GR, 512], mm_dt)

            # ---- stage 1a: q down-proj (raw) + RMS stats on local slice ----
            ssq_q = psums.tile([1, 512], F32, tag="p_sum", name="ssq_q")
            for gi, mg in enumerate(mgs):
                a2 = [pscore.tile([128, 2, 512], F32, tag="p_sc2", name="acc2")
                      for _ in range(2)]
                accs = {m: a2[j // 2][:, j % 2] for j, m in enumerate(mg)}
                for k in range(KC):
                    wt = wdq_t.pop((gi, k))
                    for j, m in enumerate(mg):
                        nc.tensor.matmul(
                            accs[m], wt[:, j * 128:(j + 1) * 128], xtl_sb[:, k, :],
                            start=(k == 0), stop=(k == KC - 1))
                for m in mg:
                    sq = spool.tile([128, 512], mm_dt, tag="sq", bufs=1)
                    nc.scalar.activation(out=sq, in_=accs[m],
                                         func=mybir.ActivationFunctionType.Square)
                    nc.tensor.matmul(ssq_q, ones_sb, sq,
                                     start=(m == 0), stop=(m == QC - 1),
                                     skip_group_check=True)
                    r = spool.tile([128, 512], mm_dt, tag="qdout%d" % (m % 2), bufs=1)
                    nc.vector.tensor_copy(r, accs[m])
                    nc.sync.dma_start(out=g_in[m * 128:(m + 1) * 128, :], in_=r)
            sd = spool.tile([1, 512], F32, tag="sdn", bufs=1)
            nc.scalar.activation(out=sd, in_=ssq_q,
                                 func=mybir.ActivationFunctionType.Sqrt,
                                 bias=eps_sb, scale=1.0 / Q_LORA)
            rstd = spool.tile([1, 512], F32, tag="rstdn", bufs=1)
            nc.vector.reciprocal(rstd, sd)
            rstd_b = spool.tile([1, 512], mm_dt, tag="rstdb")
            nc.vector.tensor_copy(rstd_b, rstd)
            # partition-broadcast via K=1 matmul (gpsimd only has the gather)
            rbc_ps = ppool.tile([128, 512], F32, tag="p_a", name="rbc_q")
            nc.tensor.matmul(rbc_ps, onesr_sb, rstd_b, start=True, stop=True)
            rstd_bcb = spool.tile([128, 512], mm_dt, tag="rstd_bcb_q", bufs=1)
            nc.vector.tensor_copy(rstd_bcb, rbc_ps)
            nc.sync.dma_start(out=g_in[Q_LORA:GR, :], in_=rstd_bcb)

            # ---- stage 1b: AllGather within batch groups (gpsimd queue) ----
            nc.gpsimd.collective_compute(
                "AllGather", mybir.AluOpType.bypass,
                replica_groups=GROUPS,
                ins=[g_in.opt()], outs=[g_out.opt()])

            # remaining weights (sync queue, behind the q-down stream)
            wkva_sb = persist.tile([128, KC, KV_LORA + 2 * D_ROPE], mm_dt, tag="wkva")
            nc.sync.dma_start(out=wkva_sb, in_=wkva.ap())
            wkvb_sb = persist.tile([128, VC, HPC, 256], mm_dt, tag="wkvb")
            nc.sync.dma_start(out=wkvb_sb, in_=wkvb.ap())
            mask_sb = persist.tile([128, 4, 512], F8, tag="masks")
            nc.sync.dma_start(out=mask_sb, in_=masks.ap())
            cosf_sb = persist.tile([D_ROPE, 4, 512], mm_dt, tag="cosf")
            sinf_sb = persist.tile([D_ROPE, 4, 512], mm_dt, tag="sinf")
            nc.sync.dma_start(out=cosf_sb,
                              in_=cos_f.ap().rearrange("d (c n) -> d c n", c=4))
            nc.sync.dma_start(out=sinf_sb,
                              in_=sin_f.ap().rearrange("d (c n) -> d c n", c=4))

            # ---- stage 1c (overlaps gather): compressed KV at full S ----
            ckv_sb = persist.tile([128, VC, 4, 512], mm_dt, tag="ckv")
            v_sb = persist.tile([128, NSK, HPC * D_V], mm_dt, tag="v")

            def kv_decompress(nch):
                for h in range(HPC):
                    acc = ppool.tile([128, 512], F32, tag="p_a", name="acc_kn")
                    for k in range(VC):
                        nc.tensor.matmul(acc, wkvb_sb[:, k, h, 0:128],
                                         ckv_sb[:, k, nch, :],
                                         start=(k == 0), stop=(k == VC - 1))
                    nc.vector.tensor_copy(
                        kf_sb[:, h, 4 * nch:4 * nch + 4, 0, :], acc)
                for j in range(4):
                    acc = ppool.tile([128, 512], F32, tag="p_a", name="acc_v")
                    for k in range(VC):
                        nc.tensor.matmul(
                            acc, ckv_sb[:, k, nch, j * 128:(j + 1) * 128],
                            wkvb_sb[:, k, :, 128:256],
                            start=(k == 0), stop=(k == VC - 1))
                    nc.vector.tensor_copy(v_sb[:, 4 * nch + j, :], acc)

            for nch in range(4):
                ssq = psums.tile([1, 512], F32, tag="p_sum", name="ssq_kv")
                a2 = [pscore.tile([128, 2, 512], F32, tag="p_sc2", name="acc2")
                      for _ in range(2)]
                accs = {m: a2[m // 2][:, m % 2] for m in range(4)}
                acc_r = ppool.tile([128, 512], F32, tag="p_a", name="acc_rope")
                accs[4] = acc_r
                if nch < 2:
                    xts = [xt2_sb[:, k, nch, :] for k in range(KC)]
                else:
                    xts = []
                    for k in range(KC):
                        xtt = xpool.tile([128, 512], mm_dt, tag="xt_s")
                        nc.sync.dma_start(
                            out=xtt,
                            in_=xt.ap()[k * 128:(k + 1) * 128,
                                        nch * 512:(nch + 1) * 512])
                        xts.append(xtt)
                for k in range(KC):
                    for m in range(5):
                        nc.tensor.matmul(
                            accs[m], wkva_sb[:, k, m * 128:(m + 1) * 128],
                            xts[k],
                            start=(k == 0), stop=(k == KC - 1))
                raw = []
                for m in range(4):
                    sq = spool.tile([128, 512], mm_dt, tag="sq", bufs=1)
                    nc.scalar.activation(out=sq, in_=accs[m],
                                         func=mybir.ActivationFunctionType.Square)
                    nc.tensor.matmul(ssq, ones_sb, sq,
                                     start=(m == 0), stop=(m == 3),
                                     skip_group_check=True)
                    r = spool.tile([128, 512], mm_dt, tag="kvraw%d" % m, bufs=1)
                    nc.vector.tensor_copy(r, accs[m])
                    raw.append((m, r))
                # rope chunk [E(64) | R(64)] -> k_pe (fp8), fanned into kf
                t0 = spool.tile([D_ROPE, 512], mm_dt, tag="ropet0", bufs=1)
                t1 = spool.tile([D_ROPE, 512], mm_dt, tag="ropet1", bufs=1)
                nc.vector.tensor_tensor(t0, acc_r[0:D_ROPE, :],
                                        cosf_sb[:, nch, :], mybir.AluOpType.mult)
                nc.vector.tensor_tensor(t1, acc_r[D_ROPE:2 * D_ROPE, :],
                                        sinf_sb[:, nch, :], mybir.AluOpType.mult)
                pe8 = spool.tile([D_ROPE, 512], F8, tag="ropeo8")
                nc.vector.tensor_tensor(pe8, t0, t1, mybir.AluOpType.add)
                for h in range(HPC):
                    nc.sync.dma_start(
                        out=kf_sb[0:64, h, 4 * nch:4 * nch + 4, 1, :],
                        in_=pe8)
                sd2 = spool.tile([1, 512], F32, tag="sdn", bufs=1)
                nc.scalar.activation(out=sd2, in_=ssq,
                                     func=mybir.ActivationFunctionType.Sqrt,
                                     bias=eps_sb, scale=1.0 / KV_LORA)
                rstd2 = spool.tile([1, 512], F32, tag="rstdn", bufs=1)
                nc.vector.reciprocal(rstd2, sd2)
                rstd2_b = spool.tile([1, 512], mm_dt, tag="rstdb")
                nc.vector.tensor_copy(rstd2_b, rstd2)
                rbc2 = ppool.tile([128, 512], F32, tag="p_a", name="rbc_kv")
                nc.tensor.matmul(rbc2, onesr_sb, rstd2_b, start=True, stop=True)
                for m, r in raw:
                    nc.vector.tensor_tensor(ckv_sb[:, m, nch, :], r, rbc2,
                                            mybir.AluOpType.mult)
                # decompress one chunk behind so kn/v also run under the
                # gather instead of serializing after it
                if nch > 0:
                    kv_decompress(nch - 1)
            kv_decompress(3)

            # ---- stage 2b/2c/2d: per-seq-chunk q up-proj, attention, o ----
            for sqc in range(4):
                qnorm_t = [spool.tile([128, 512], mm_dt, tag="qn_stream%d" % (k % 4),
                                      name="qnorm_t", bufs=3) for k in range(QC)]
                for k in range(QC):
                    nc.sync.dma_start(
                        out=qnorm_t[k],
                        in_=g_out[sqc * GR + k * 128: sqc * GR + (k + 1) * 128, :])
                rstd_t = spool.tile([128, 512], mm_dt, tag="rstd_t", bufs=1)
                nc.sync.dma_start(out=rstd_t,
                                  in_=g_out[sqc * GR + Q_LORA: (sqc + 1) * GR, :])
                for h in range(HPC):
                    wts = []
                    for k in range(QC):
                        wt = wpool.tile([128, 256], mm_dt, tag="w_uq")
                        nc.sync.dma_start(
                            out=wt, in_=wuq.ap()[:, k, h * 256:(h + 1) * 256])
                        wts.append(wt)
                    acc2 = pscore.tile([128, 2, 512], F32, tag="p_sc2", name="acc_qup")
                    for k in range(QC):
                        for j in range(2):
                            nc.tensor.matmul(
                                acc2[:, j], wts[k][:, j * 128:(j + 1) * 128],
                                qnorm_t[k],
                                start=(k == 0), stop=(k == QC - 1))
                    # fp8 q pack [nope | rope]; RMS rstd folded in here
                    qf = qf_t[h]
                    nc.vector.tensor_tensor(qf[:, 0, :], acc2[:, 0], rstd_t,
                                            mybir.AluOpType.mult)
                    t0 = spool.tile([D_ROPE, 512], mm_dt, tag="ropet0", bufs=1)
                    t1 = spool.tile([D_ROPE, 512], mm_dt, tag="ropet1", bufs=1)
                    nc.vector.tensor_tensor(t0, acc2[0:D_ROPE, 1], cosf_sb[:, sqc, :],
                                            mybir.AluOpType.mult)
                    nc.vector.tensor_tensor(t1, acc2[D_ROPE:2 * D_ROPE, 1],
                                            sinf_sb[:, sqc, :], mybir.AluOpType.mult)
                    t2 = spool.tile([D_ROPE, 512], mm_dt, tag="ropeo")
                    nc.vector.tensor_tensor(t2, t0, t1, mybir.AluOpType.add)
                    nc.vector.tensor_tensor(qf[0:64, 1, :], t2, rstd_t[0:64, :],
                                            mybir.AluOpType.mult)
                    qf_t[h] = qf

                n_skt = 4 * (sqc + 1)
                ctx_sb = spool.tile([D_V, HPC, 512], mm_dt, tag="ctx", bufs=2)
                for h in range(HPC):
                    sum_acc = psums.tile([1, 512], F32, tag="p_sum", name="sum_acc")
                    ctx_acc = pctx.tile([D_V, 512], F32, tag="p_ctx")

                    def drain(pex2, pskp):
                        for half in range(2):
                            skt = 2 * pskp + half
                            pd = skt - 4 * sqc
                            c0 = 128 * pd if pd > 0 else 0
                            pex = pex2[:, half]
                            nc.tensor.matmul(sum_acc[:, c0:], ones_sb,
                                             pex[:, c0:],
                                             start=(skt == 0),
                                             stop=(skt == n_skt - 1),
                                             skip_group_check=True)
                            nc.tensor.matmul(ctx_acc[:, c0:],
                                             v_sb[:, skt, h * D_V:(h + 1) * D_V],
                                             pex[:, c0:],
                                             start=(skt == 0),
                                             stop=(skt == n_skt - 1),
                                             skip_group_check=True)

                    pending = None   # software pipeline: exp pair awaiting PV
                    for skp in range(n_skt // 2):
                        sc2 = pscore.tile([128, 2, 512], F32, tag="p_sc2",
                                          name="sc2")
                        for half in range(2):
                            skt = 2 * skp + half
                            nc.tensor.matmul(
                                sc2[:, half], kf_sb[:, h, skt], qf_t[h],
                                start=True, stop=True, perf_mode=DR,
                                skip_group_check=True)
                        ex2 = spool.tile([128, 2, 512], mm_dt,
                                         tag="exp%d" % (skp % 2), bufs=2)
                        nc.scalar.activation(out=ex2, in_=sc2,
                                             func=mybir.ActivationFunctionType.Exp,
                                             scale=SCALE)
                        d0 = 2 * skp - 4 * sqc
                        if d0 >= 0:
                            nc.vector.tensor_tensor(ex2, ex2,
                                                    mask_sb[:, d0:d0 + 2, :],
                                                    mybir.AluOpType.mult)
                        if pending is not None:
                            drain(*pending)
                        pending = (ex2, skp)
                    drain(*pending)
                    # raw evacuation frees the single ctx bank immediately
                    ctxr = spool.tile([D_V, 512], mm_dt, tag="ctxr%d" % (h % 2), bufs=1)
                    nc.vector.tensor_copy(ctxr, ctx_acc)
                    # 1/sum via exp(-ln(sum)) on ScalarE
                    ls = spool.tile([1, 512], F32, tag="lsum", bufs=1)
                    nc.scalar.activation(out=ls, in_=sum_acc,
                                         func=mybir.ActivationFunctionType.Ln)
                    rc = spool.tile([1, 512], F32, tag="recip1", bufs=1)
                    nc.scalar.activation(out=rc, in_=ls, scale=-1.0,
                                         func=mybir.ActivationFunctionType.Exp)
                    rb = spool.tile([128, 512], F32, tag="recip_bc", bufs=1)
                    nc.gpsimd.partition_broadcast(rb, rc)
                    nc.vector.tensor_tensor(ctx_sb[:, h, :], ctxr, rb,
                                            mybir.AluOpType.mult)

                # ---- output projection for this seq chunk ----
                for hidc in range(HID // 128):
                    owt = wpool.tile([D_V, HPC, 128], mm_dt, tag="w_o")
                    nc.sync.dma_start(
                        out=owt, in_=ow.ap()[:, :, hidc * 128:(hidc + 1) * 128])
                    acc = ppool.tile([128, 512], F32, tag="p_a", name="acc_o")
                    for h in range(HPC):
                        nc.tensor.matmul(acc, owt[:, h, :], ctx_sb[:, h, :],
                                         start=(h == 0), stop=(h == HPC - 1))
                    o = spool.tile([128, 512], mm_dt, tag="oout", bufs=1)
                    nc.vector.tensor_copy(o, acc)
                    nc.gpsimd.dma_start(
                        out=out_t.ap()[hidc * 128:(hidc + 1) * 128,
                                       sqc * 512:(sqc + 1) * 512],
                        in_=o)

    nc.compile()
    return nc


# ------------------------------------------------------------- host side --
def _rope_tables():
    inv_freq = 1.0 / (ROPE_THETA ** (np.arange(0, D_ROPE, 2, dtype=np.float64) / D_ROPE))
    t = np.arange(S, dtype=np.float64)
    freqs = np.outer(t, inv_freq)                    # [S, 32]
    emb = np.concatenate([freqs, freqs], axis=-1)    # [S, 64]
    return (np.cos(emb).astype(np.float32).T.copy(),
            np.sin(emb).astype(np.float32).T.copy())  # [64, S]


_E_PERM = np.concatenate([np.arange(0, D_ROPE, 2), np.arange(1, D_ROPE, 2)])


def _rope_expand(Wpe):
    """[n, 64] rope weight cols -> [n, 128]: [even/odd-reordered | rot-half signed]."""
    Y = Wpe[:, _E_PERM]
    R = np.concatenate([-Y[:, D_ROPE // 2:], Y[:, :D_ROPE // 2]], axis=1)
    return np.concatenate([Y, R], axis=1)


def _chunk_rows(a, p=128):
    """[R, ...] -> [p, R//p, ...] grouping rows into chunks of p."""
    R, Cs = a.shape[0], a.shape[1:]
    return np.ascontiguousarray(a.reshape(R // p, p, *Cs).transpose(
        1, 0, *range(2, a.ndim + 1)))


def _prep_inputs(hidden_states, w_dq, q_a_ln_w, w_uq, kv_a_w, kv_a_ln_w, kv_b_w, o_w):
    bf = ml_dtypes.bfloat16
    s_loc = S // 4
    cosT, sinT = _rope_tables()

    wuq_eff = (np.asarray(q_a_ln_w)[:, None] * np.asarray(w_uq)).reshape(Q_LORA, H, D_Q)
    head_blocks = []
    for h in range(H):
        head_blocks.append(np.concatenate(
            [wuq_eff[:, h, :D_NOPE], _rope_expand(wuq_eff[:, h, D_NOPE:])], axis=1))
    wuq_x = np.stack(head_blocks, axis=1)            # [1536, 16, 256]

    kv_a = np.asarray(kv_a_w)
    wkva_x = np.concatenate([kv_a[:, :KV_LORA], _rope_expand(kv_a[:, KV_LORA:])],
                            axis=1).astype(bf)       # [2048, 640]
    wkva_p = _chunk_rows(wkva_x)                     # [128, 16, 640]
    wkvb_eff = (np.asarray(kv_a_ln_w)[:, None] * np.asarray(kv_b_w)).reshape(KV_LORA, H, 256)
    ow_r = np.asarray(o_w).reshape(H, D_V, HID)

    c_idx = np.arange(512)[None, :]
    r_idx = np.arange(128)[:, None]
    masks = np.stack([(c_idx >= 128 * dd + r_idx) for dd in range(4)],
                     axis=1).astype(ml_dtypes.float8_e4m3fn)      # [128, 4, 512]

    wdq_b = np.asarray(w_dq).astype(bf)
    hs = np.asarray(hidden_states)

    in_maps = []
    for c in range(N_CORES):
        b, hg = c // 4, c % 4
        s0 = 512 * hg
        xt_full = np.ascontiguousarray(hs[b].T).astype(bf)
        wuq_c = wuq_x[:, HPC * hg: HPC * (hg + 1), :].reshape(
            Q_LORA, HPC * 256).astype(bf)
        wkvb_c = wkvb_eff[:, HPC * hg: HPC * (hg + 1), :].astype(bf)
        in_maps.append({
            "xt": xt_full,
            "xt_loc": np.ascontiguousarray(xt_full[:, s0:s0 + s_loc]),
            "wdq": wdq_b,
            "wuq": _chunk_rows(wuq_c),               # [128, 12, 1024]
            "wkva": wkva_p,
            "wkvb": _chunk_rows(wkvb_c),             # [128, 4, 4, 256]
            "ow": np.ascontiguousarray(
                ow_r[HPC * hg: HPC * (hg + 1)].transpose(1, 0, 2)).astype(bf),
            "cos_f": cosT.astype(bf),
            "sin_f": sinT.astype(bf),
            "masks": masks,
        })
    return in_maps


def _postprocess(results):
    out = np.empty((B, S, HID), dtype=np.float32)
    for b in range(B):
        acc = results[4 * b]["out_t"].astype(np.float32)
        for c in GROUPS[b][1:]:
            acc = acc + results[c]["out_t"].astype(np.float32)
        out[b] = acc.T
    return out


def kernel(**inputs):
    key = (str(MM_DT),)
    if key not in _CACHE:
        _CACHE[key] = build_kernel(MM_DT)
    nc = _CACHE[key]
    in_maps = _prep_inputs(**inputs)
    r = run_bass_kernel_spmd(nc, in_maps, core_ids=list(range(N_CORES)))
    return _postprocess(r.results)


# revision 40
# speedup vs baseline: 1.0051x; 1.0051x over previous
"""DeepseekV2 MLA attention prefill kernel for 8 Trainium2 NeuronCores.

Sharding: 2-way data-parallel over batch x 4-way tensor-parallel over heads
(4 heads per core).  The raw q down-projection (+ rstd of its RMS norm) is
computed on an S/4 slice per core and exchanged with one in-group AllGather;
the RMS normalization is folded into the q up-projection output after the
gather.  The compressed-KV path is replicated at full S on every core and
computed while the gather is in flight.  Per-head up-projections, attention
and the output projection are computed locally; o_proj partial sums are
reduced on the host during unsharding.

Key scheduling/efficiency points (v4):
 - score matmuls run in fp8e4 DoubleRow: the two 128-deep k-subtiles are
   [k_nope | (k_pe ; zeros)], so one PE pass per 128x512 score block covers
   the full 192-dim contraction (rope included); q/k packs are built by the
   DVE/DMA on the side.  Everything else stays bf16 (fp8 there fails the
   2e-2 tolerance; scores measured 1.0e-2 in emulation).
 - the collective lives alone on the gpsimd queue; RMS rstd broadcasts are
   done by a K=1 PE matmul against a ones row so the kv-norm never blocks
   behind the 90us gather.
 - DMA priority at startup: only the q-down critical stream (xt_loc + wdq)
   is issued first; all other weights follow it on the same queue.
 - exp is evaluated over [128, 1024] pairs of score banks (halves ScalarE
   instruction overhead); attention context is evacuated raw and the
   softmax 1/sum is applied during a later DVE pass, so the single-bank
   ctx accumulator frees immediately at head boundaries.
 - PV and row-sum matmuls restrict their free dim on diagonal blocks.

Layouts: activations are feature-major ([D, S]); scores are computed
transposed ([s_k, s_q]) so PV needs no transposes.  RoPE uses host-side
permuted/sign-folded weight columns.  PSUM accumulation fp32 throughout.
"""
import sys
sys.path.insert(0, "/opt/trn_rl_repo")

import math
import numpy as np
import ml_dtypes

import concourse.bass as bass
import concourse.tile as tile
from concourse import bacc, mybir
from concourse.bass_utils import run_bass_kernel_spmd

# ---- problem constants (hardcoded; kernel.py must be self-contained) ----
B, S, HID, H = 2, 2048, 2048, 16
Q_LORA, KV_LORA = 1536, 512
D_NOPE, D_ROPE, D_V = 128, 64, 128
D_Q = D_NOPE + D_ROPE
EPS = 1e-6
ROPE_THETA = 10000.0
N_CORES = 8
HPC = 4                      # heads per core
GROUPS = [[0, 1, 2, 3], [4, 5, 6, 7]]

KC = HID // 128              # 16
QC = Q_LORA // 128           # 12
VC = KV_LORA // 128          # 4
NSK = S // 128               # 16 key blocks

F32 = mybir.dt.float32
BF16 = mybir.dt.bfloat16
F8 = mybir.dt.float8e4
MM_DT = BF16
DR = mybir.MatmulPerfMode.DoubleRow

SCALE = 1.0 / math.sqrt(D_Q)

_CACHE = {}


# ---------------------------------------------------------------- builder --
def build_kernel(mm_dt=MM_DT):
    s_loc = S // 4

    nc = bacc.Bacc("TRN2", target_bir_lowering=False, debug=False,
                   num_devices=N_CORES)

    xt = nc.dram_tensor("xt", [HID, S], mm_dt, kind="ExternalInput")
    xt_loc = nc.dram_tensor("xt_loc", [HID, s_loc], mm_dt, kind="ExternalInput")
    wdq = nc.dram_tensor("wdq", [HID, Q_LORA], mm_dt, kind="ExternalInput")
    wuq = nc.dram_tensor("wuq", [128, QC, HPC * 256], mm_dt, kind="ExternalInput")
    wkva = nc.dram_tensor("wkva", [128, KC, KV_LORA + 2 * D_ROPE], mm_dt,
                          kind="ExternalInput")
    wkvb = nc.dram_tensor("wkvb", [128, VC, HPC, 256], mm_dt, kind="ExternalInput")
    ow = nc.dram_tensor("ow", [D_V, HPC, HID], mm_dt, kind="ExternalInput")
    cos_f = nc.dram_tensor("cos_f", [D_ROPE, S], mm_dt, kind="ExternalInput")
    sin_f = nc.dram_tensor("sin_f", [D_ROPE, S], mm_dt, kind="ExternalInput")
    masks = nc.dram_tensor("masks", [128, 4, 512], mm_dt, kind="ExternalInput")
    out_t = nc.dram_tensor("out_t", [HID, S], mm_dt, kind="ExternalOutput")

    with tile.TileContext(nc) as tc:
        import contextlib
        ctx = contextlib.ExitStack()
        with ctx:
            persist = ctx.enter_context(tc.tile_pool(name="persist", bufs=1))
            wpool = ctx.enter_context(tc.tile_pool(name="wpool", bufs=3))
            spool = ctx.enter_context(tc.tile_pool(name="spool", bufs=2))
            xpool = ctx.enter_context(tc.tile_pool(name="xpool", bufs=3))
            # PSUM: ppool 2 + pscore 2x2banks + pctx 1 + psums 1 = 8 banks
            ppool = ctx.enter_context(tc.tile_pool(name="ppool", bufs=2, space="PSUM"))
            pscore = ctx.enter_context(tc.tile_pool(name="pscore", bufs=2, space="PSUM"))
            pctx = ctx.enter_context(tc.tile_pool(name="pctx", bufs=1, space="PSUM"))
            psums = ctx.enter_context(tc.tile_pool(name="psums", bufs=1, space="PSUM"))
            dram = ctx.enter_context(tc.tile_pool(name="dram", bufs=1, space="DRAM"))

            ones_sb = persist.tile([128, 1], mm_dt, tag="ones")
            nc.vector.memset(ones_sb, 1.0)
            onesr_sb = persist.tile([1, 128], mm_dt, tag="onesr")
            nc.vector.memset(onesr_sb, 1.0)
            eps_sb = persist.tile([1, 1], F32, tag="eps")
            nc.vector.memset(eps_sb, EPS)

            # fused fp8 key pack: [d(128), h, skt, {nope | rope}, s_k(128)];
            # rope rows 64-127 are zero so the q-side values there are inert
            kf_sb = persist.tile([128, HPC, NSK, 2, 128], F8, tag="kf")
            nc.vector.memset(kf_sb[64:128, :, :, 1, :], 0.0)
            # fp8 q packs: rows 64-127 of the rope subtile are never written,
            # and uninitialized fp8 bytes can decode as NaN (NaN*0=NaN in the
            # PE), so zero them once up front.
            qf_t = {}
            for h in range(HPC):
                qf_t[h] = persist.tile([128, 2, 512], F8, tag="qf_h%d" % h,
                                       name="qf%d" % h)
                nc.vector.memset(qf_t[h][64:128, 1, :], 0.0)

            # ---- q-down critical DMA stream first: xt_loc + wdq ----
            xtl_sb = persist.tile([128, KC, 512], mm_dt, tag="xtl")
            mgs = [list(range(g, g + 4)) for g in range(0, QC, 4)]
            wdq_t = {}
            for k in range(KC):
                nc.sync.dma_start(out=xtl_sb[:, k, :],
                                  in_=xt_loc.ap()[k * 128:(k + 1) * 128, :])
                wt = wpool.tile([128, 512], mm_dt, tag="w_s1", bufs=6)
                nc.sync.dma_start(
                    out=wt, in_=wdq.ap()[k * 128:(k + 1) * 128, 0:512])
                wdq_t[(0, k)] = wt
            for gi in range(1, 3):
                for k in range(KC):
                    wt = wpool.tile([128, 512], mm_dt, tag="w_s1", bufs=6)
                    nc.sync.dma_start(
                        out=wt,
                        in_=wdq.ap()[k * 128:(k + 1) * 128,
                                     gi * 512:(gi + 1) * 512])
                    wdq_t[(gi, k)] = wt

            # gather buffers (DRAM): 12 raw q-down chunks + broadcast rstd
            GR = Q_LORA + 128
            g_in = dram.tile([GR, s_loc], mm_dt)
            g_out = dram.tile([4 * GR, 512], mm_dt)

            # ---- stage 1a: q down-proj (raw) + RMS stats on local slice ----
            ssq_q = psums.tile([1, 512], F32, tag="p_sum", name="ssq_q")
            for gi, mg in enumerate(mgs):
                a2 = [pscore.tile([128, 2, 512], F32, tag="p_sc2", name="acc2")
                      for _ in range(2)]
                accs = {m: a2[j // 2][:, j % 2] for j, m in enumerate(mg)}
                for k in range(KC):
                    wt = wdq_t.pop((gi, k))
                    for j, m in enumerate(mg):
                        nc.tensor.matmul(
                            accs[m], wt[:, j * 128:(j + 1) * 128], xtl_sb[:, k, :],
                            start=(k == 0), stop=(k == KC - 1))
                for m in mg:
                    sq = spool.tile([128, 512], mm_dt, tag="sq")
                    nc.scalar.activation(out=sq, in_=accs[m],
                                         func=mybir.ActivationFunctionType.Square)
                    nc.tensor.matmul(ssq_q, ones_sb, sq,
                                     start=(m == 0), stop=(m == QC - 1),
                                     skip_group_check=True)
                    r = spool.tile([128, 512], mm_dt, tag="qdout%d" % (m % 4))
                    nc.vector.tensor_copy(r, accs[m])
                    nc.sync.dma_start(out=g_in[m * 128:(m + 1) * 128, :], in_=r)
            sd = spool.tile([1, 512], F32, tag="sdn")
            nc.scalar.activation(out=sd, in_=ssq_q,
                                 func=mybir.ActivationFunctionType.Sqrt,
                                 bias=eps_sb, scale=1.0 / Q_LORA)
            rstd = spool.tile([1, 512], F32, tag="rstdn")
            nc.vector.reciprocal(rstd, sd)
            rstd_b = spool.tile([1, 512], mm_dt, tag="rstdb")
            nc.vector.tensor_copy(rstd_b, rstd)
            # partition-broadcast via K=1 matmul (gpsimd only has the gather)
            rbc_ps = ppool.tile([128, 512], F32, tag="p_a", name="rbc_q")
            nc.tensor.matmul(rbc_ps, onesr_sb, rstd_b, start=True, stop=True)
            rstd_bcb = spool.tile([128, 512], mm_dt, tag="rstd_bcb_q", bufs=1)
            nc.vector.tensor_copy(rstd_bcb, rbc_ps)
            nc.sync.dma_start(out=g_in[Q_LORA:GR, :], in_=rstd_bcb)

            # ---- stage 1b: AllGather within batch groups (gpsimd queue) ----
            nc.gpsimd.collective_compute(
                "AllGather", mybir.AluOpType.bypass,
                replica_groups=GROUPS,
                ins=[g_in.opt()], outs=[g_out.opt()])

            # remaining weights (sync queue, behind the q-down stream)
            wkva_sb = persist.tile([128, KC, KV_LORA + 2 * D_ROPE], mm_dt, tag="wkva")
            nc.sync.dma_start(out=wkva_sb, in_=wkva.ap())
            wkvb_sb = persist.tile([128, VC, HPC, 256], mm_dt, tag="wkvb")
            nc.sync.dma_start(out=wkvb_sb, in_=wkvb.ap())
            mask_sb = persist.tile([128, 4, 512], mm_dt, tag="masks")
            nc.sync.dma_start(out=mask_sb, in_=masks.ap())
            cosf_sb = persist.tile([D_ROPE, 4, 512], mm_dt, tag="cosf")
            sinf_sb = persist.tile([D_ROPE, 4, 512], mm_dt, tag="sinf")
            nc.sync.dma_start(out=cosf_sb,
                              in_=cos_f.ap().rearrange("d (c n) -> d c n", c=4))
            nc.sync.dma_start(out=sinf_sb,
                              in_=sin_f.ap().rearrange("d (c n) -> d c n", c=4))

            # ---- stage 1c (overlaps gather): compressed KV at full S ----
            ckv_sb = persist.tile([128, VC, 4, 512], mm_dt, tag="ckv")
            for nch in range(4):
                ssq = psums.tile([1, 512], F32, tag="p_sum", name="ssq_kv")
                a2 = [pscore.tile([128, 2, 512], F32, tag="p_sc2", name="acc2")
                      for _ in range(2)]
                accs = {m: a2[m // 2][:, m % 2] for m in range(4)}
                acc_r = ppool.tile([128, 512], F32, tag="p_a", name="acc_rope")
                accs[4] = acc_r
                for k in range(KC):
                    xtt = xpool.tile([128, 512], mm_dt, tag="xt_s")
                    nc.sync.dma_start(
                        out=xtt,
                        in_=xt.ap()[k * 128:(k + 1) * 128,
                                    nch * 512:(nch + 1) * 512])
                    for m in range(5):
                        nc.tensor.matmul(
                            accs[m], wkva_sb[:, k, m * 128:(m + 1) * 128], xtt,
                            start=(k == 0), stop=(k == KC - 1))
                raw = []
                for m in range(4):
                    sq = spool.tile([128, 512], mm_dt, tag="sq")
                    nc.scalar.activation(out=sq, in_=accs[m],
                                         func=mybir.ActivationFunctionType.Square)
                    nc.tensor.matmul(ssq, ones_sb, sq,
                                     start=(m == 0), stop=(m == 3),
                                     skip_group_check=True)
                    r = spool.tile([128, 512], mm_dt, tag="kvraw%d" % m, bufs=1)
                    nc.vector.tensor_copy(r, accs[m])
                    raw.append((m, r))
                # rope chunk [E(64) | R(64)] -> k_pe (fp8), fanned into kf
                t0 = spool.tile([D_ROPE, 512], mm_dt, tag="ropet0")
                t1 = spool.tile([D_ROPE, 512], mm_dt, tag="ropet1")
                nc.vector.tensor_tensor(t0, acc_r[0:D_ROPE, :],
                                        cosf_sb[:, nch, :], mybir.AluOpType.mult)
                nc.vector.tensor_tensor(t1, acc_r[D_ROPE:2 * D_ROPE, :],
                                        sinf_sb[:, nch, :], mybir.AluOpType.mult)
                pe8 = spool.tile([D_ROPE, 512], F8, tag="ropeo8")
                nc.vector.tensor_tensor(pe8, t0, t1, mybir.AluOpType.add)
                for h in range(HPC):
                    nc.sync.dma_start(
                        out=kf_sb[0:64, h, 4 * nch:4 * nch + 4, 1, :],
                        in_=pe8)
                sd2 = spool.tile([1, 512], F32, tag="sdn")
                nc.scalar.activation(out=sd2, in_=ssq,
                                     func=mybir.ActivationFunctionType.Sqrt,
                                     bias=eps_sb, scale=1.0 / KV_LORA)
                rstd2 = spool.tile([1, 512], F32, tag="rstdn")
                nc.vector.reciprocal(rstd2, sd2)
                rstd2_b = spool.tile([1, 512], mm_dt, tag="rstdb")
                nc.vector.tensor_copy(rstd2_b, rstd2)
                rbc2 = ppool.tile([128, 512], F32, tag="p_a", name="rbc_kv")
                nc.tensor.matmul(rbc2, onesr_sb, rstd2_b, start=True, stop=True)
                for m, r in raw:
                    nc.vector.tensor_tensor(ckv_sb[:, m, nch, :], r, rbc2,
                                            mybir.AluOpType.mult)

            # ---- stage 2a: decompress KV (full S, local heads) ----
            for h in range(HPC):
                for skc in range(4):
                    acc = ppool.tile([128, 512], F32, tag="p_a", name="acc_kn")
                    for k in range(VC):
                        nc.tensor.matmul(acc, wkvb_sb[:, k, h, 0:128],
                                         ckv_sb[:, k, skc, :],
                                         start=(k == 0), stop=(k == VC - 1))
                    nc.vector.tensor_copy(
                        kf_sb[:, h, 4 * skc:4 * skc + 4, 0, :], acc)

            v_sb = persist.tile([128, NSK, HPC * D_V], mm_dt, tag="v")
            for skt in range(NSK):
                acc = ppool.tile([128, 512], F32, tag="p_a", name="acc_v")
                for k in range(VC):
                    nc.tensor.matmul(
                        acc,
                        ckv_sb[:, k, skt // 4, (skt % 4) * 128:(skt % 4) * 128 + 128],
                        wkvb_sb[:, k, :, 128:256],
                        start=(k == 0), stop=(k == VC - 1))
                nc.vector.tensor_copy(v_sb[:, skt, :], acc)

            # ---- stage 2b/2c/2d: per-seq-chunk q up-proj, attention, o ----
            for sqc in range(4):
                qnorm_t = [spool.tile([128, 512], mm_dt, tag="qn_stream%d" % (k % 4),
                                      name="qnorm_t", bufs=3) for k in range(QC)]
                for k in range(QC):
                    nc.sync.dma_start(
                        out=qnorm_t[k],
                        in_=g_out[sqc * GR + k * 128: sqc * GR + (k + 1) * 128, :])
                rstd_t = spool.tile([128, 512], mm_dt, tag="rstd_t", bufs=2)
                nc.sync.dma_start(out=rstd_t,
                                  in_=g_out[sqc * GR + Q_LORA: (sqc + 1) * GR, :])
                for h in range(HPC):
                    wts = []
                    for k in range(QC):
                        wt = wpool.tile([128, 256], mm_dt, tag="w_uq")
                        nc.sync.dma_start(
                            out=wt, in_=wuq.ap()[:, k, h * 256:(h + 1) * 256])
                        wts.append(wt)
                    acc2 = pscore.tile([128, 2, 512], F32, tag="p_sc2", name="acc_qup")
                    for k in range(QC):
                        for j in range(2):
                            nc.tensor.matmul(
                                acc2[:, j], wts[k][:, j * 128:(j + 1) * 128],
                                qnorm_t[k],
                                start=(k == 0), stop=(k == QC - 1))
                    # fp8 q pack [nope | rope]; RMS rstd folded in here
                    qf = qf_t[h]
                    nc.vector.tensor_tensor(qf[:, 0, :], acc2[:, 0], rstd_t,
                                            mybir.AluOpType.mult)
                    t0 = spool.tile([D_ROPE, 512], mm_dt, tag="ropet0")
                    t1 = spool.tile([D_ROPE, 512], mm_dt, tag="ropet1")
                    nc.vector.tensor_tensor(t0, acc2[0:D_ROPE, 1], cosf_sb[:, sqc, :],
                                            mybir.AluOpType.mult)
                    nc.vector.tensor_tensor(t1, acc2[D_ROPE:2 * D_ROPE, 1],
                                            sinf_sb[:, sqc, :], mybir.AluOpType.mult)
                    t2 = spool.tile([D_ROPE, 512], mm_dt, tag="ropeo")
                    nc.vector.tensor_tensor(t2, t0, t1, mybir.AluOpType.add)
                    nc.vector.tensor_tensor(qf[0:64, 1, :], t2, rstd_t[0:64, :],
                                            mybir.AluOpType.mult)
                    qf_t[h] = qf

                n_skt = 4 * (sqc + 1)
                ctx_sb = spool.tile([D_V, HPC, 512], mm_dt, tag="ctx", bufs=2)
                for h in range(HPC):
                    sum_acc = psums.tile([1, 512], F32, tag="p_sum", name="sum_acc")
                    ctx_acc = pctx.tile([D_V, 512], F32, tag="p_ctx")

                    def drain(pex2, pskp):
                        for half in range(2):
                            skt = 2 * pskp + half
                            pd = skt - 4 * sqc
                            c0 = 128 * pd if pd > 0 else 0
                            pex = pex2[:, half]
                            nc.tensor.matmul(sum_acc[:, c0:], ones_sb,
                                             pex[:, c0:],
                                             start=(skt == 0),
                                             stop=(skt == n_skt - 1),
                                             skip_group_check=True)
                            nc.tensor.matmul(ctx_acc[:, c0:],
                                             v_sb[:, skt, h * D_V:(h + 1) * D_V],
                                             pex[:, c0:],
                                             start=(skt == 0),
                                             stop=(skt == n_skt - 1),
                                             skip_group_check=True)

                    pending = None   # software pipeline: exp pair awaiting PV
                    for skp in range(n_skt // 2):
                        sc2 = pscore.tile([128, 2, 512], F32, tag="p_sc2",
                                          name="sc2")
                        for half in range(2):
                            skt = 2 * skp + half
                            nc.tensor.matmul(
                                sc2[:, half], kf_sb[:, h, skt], qf_t[h],
                                start=True, stop=True, perf_mode=DR,
                                skip_group_check=True)
                        ex2 = spool.tile([128, 2, 512], mm_dt,
                                         tag="exp%d" % (skp % 2), bufs=2)
                        nc.scalar.activation(out=ex2, in_=sc2,
                                             func=mybir.ActivationFunctionType.Exp,
                                             scale=SCALE)
                        d0 = 2 * skp - 4 * sqc
                        if d0 >= 0:
                            nc.vector.tensor_tensor(ex2, ex2,
                                                    mask_sb[:, d0:d0 + 2, :],
                                                    mybir.AluOpType.mult)
                        if pending is not None:
                            drain(*pending)
                        pending = (ex2, skp)
                    drain(*pending)
                    # raw evacuation frees the single ctx bank immediately
                    ctxr = spool.tile([D_V, 512], mm_dt, tag="ctxr%d" % h, bufs=1)
                    nc.vector.tensor_copy(ctxr, ctx_acc)
                    # 1/sum via exp(-ln(sum)) on ScalarE
                    ls = spool.tile([1, 512], F32, tag="lsum")
                    nc.scalar.activation(out=ls, in_=sum_acc,
                                         func=mybir.ActivationFunctionType.Ln)
                    rc = spool.tile([1, 512], F32, tag="recip1")
                    nc.scalar.activation(out=rc, in_=ls, scale=-1.0,
                                         func=mybir.ActivationFunctionType.Exp)
                    rb = spool.tile([128, 512], F32, tag="recip_bc")
                    nc.gpsimd.partition_broadcast(rb, rc)
                    nc.vector.tensor_tensor(ctx_sb[:, h, :], ctxr, rb,
                                            mybir.AluOpType.mult)

                # ---- output projection for this seq chunk ----
                for hidc in range(HID // 128):
                    owt = wpool.tile([D_V, HPC, 128], mm_dt, tag="w_o")
                    nc.sync.dma_start(
                        out=owt, in_=ow.ap()[:, :, hidc * 128:(hidc + 1) * 128])
                    acc = ppool.tile([128, 512], F32, tag="p_a", name="acc_o")
                    for h in range(HPC):
                        nc.tensor.matmul(acc, owt[:, h, :], ctx_sb[:, h, :],
                                         start=(h == 0), stop=(h == HPC - 1))
                    o = spool.tile([128, 512], mm_dt, tag="oout")
                    nc.vector.tensor_copy(o, acc)
                    nc.gpsimd.dma_start(
                        out=out_t.ap()[hidc * 128:(hidc + 1) * 128,
                                       sqc * 512:(sqc + 1) * 512],
                        in_=o)

    nc.compile()
    return nc


# ------------------------------------------------------------- host side --
def _rope_tables():
    inv_freq = 1.0 / (ROPE_THETA ** (np.arange(0, D_ROPE, 2, dtype=np.float64) / D_ROPE))
    t = np.arange(S, dtype=np.float64)
    freqs = np.outer(t, inv_freq)                    # [S, 32]
    emb = np.concatenate([freqs, freqs], axis=-1)    # [S, 64]
    return (np.cos(emb).astype(np.float32).T.copy(),
            np.sin(emb).astype(np.float32).T.copy())  # [64, S]


_E_PERM = np.concatenate([np.arange(0, D_ROPE, 2), np.arange(1, D_ROPE, 2)])


def _rope_expand(Wpe):
    """[n, 64] rope weight cols -> [n, 128]: [even/odd-reordered | rot-half signed]."""
    Y = Wpe[:, _E_PERM]
    R = np.concatenate([-Y[:, D_ROPE // 2:], Y[:, :D_ROPE // 2]], axis=1)
    return np.concatenate([Y, R], axis=1)


def _chunk_rows(a, p=128):
    """[R, ...] -> [p, R//p, ...] grouping rows into chunks of p."""
    R, Cs = a.shape[0], a.shape[1:]
    return np.ascontiguousarray(a.reshape(R // p, p, *Cs).transpose(
        1, 0, *range(2, a.ndim + 1)))


def _prep_inputs(hidden_states, w_dq, q_a_ln_w, w_uq, kv_a_w, kv_a_ln_w, kv_b_w, o_w):
    bf = ml_dtypes.bfloat16
    s_loc = S // 4
    cosT, sinT = _rope_tables()

    wuq_eff = (np.asarray(q_a_ln_w)[:, None] * np.asarray(w_uq)).reshape(Q_LORA, H, D_Q)
    head_blocks = []
    for h in range(H):
        head_blocks.append(np.concatenate(
            [wuq_eff[:, h, :D_NOPE], _rope_expand(wuq_eff[:, h, D_NOPE:])], axis=1))
    wuq_x = np.stack(head_blocks, axis=1)            # [1536, 16, 256]

    kv_a = np.asarray(kv_a_w)
    wkva_x = np.concatenate([kv_a[:, :KV_LORA], _rope_expand(kv_a[:, KV_LORA:])],
                            axis=1).astype(bf)       # [2048, 640]
    wkva_p = _chunk_rows(wkva_x)                     # [128, 16, 640]
    wkvb_eff = (np.asarray(kv_a_ln_w)[:, None] * np.asarray(kv_b_w)).reshape(KV_LORA, H, 256)
    ow_r = np.asarray(o_w).reshape(H, D_V, HID)

    c_idx = np.arange(512)[None, :]
    r_idx = np.arange(128)[:, None]
    masks = np.stack([(c_idx >= 128 * dd + r_idx) for dd in range(4)],
                     axis=1).astype(bf)              # [128, 4, 512]

    wdq_b = np.asarray(w_dq).astype(bf)
    hs = np.asarray(hidden_states)

    in_maps = []
    for c in range(N_CORES):
        b, hg = c // 4, c % 4
        s0 = 512 * hg
        xt_full = np.ascontiguousarray(hs[b].T).astype(bf)
        wuq_c = wuq_x[:, HPC * hg: HPC * (hg + 1), :].reshape(
            Q_LORA, HPC * 256).astype(bf)
        wkvb_c = wkvb_eff[:, HPC * hg: HPC * (hg + 1), :].astype(bf)
        in_maps.append({
            "xt": xt_full,
            "xt_loc": np.ascontiguousarray(xt_full[:, s0:s0 + s_loc]),
            "wdq": wdq_b,
            "wuq": _chunk_rows(wuq_c),               # [128, 12, 1024]
            "wkva": wkva_p,
            "wkvb": _chunk_rows(wkvb_c),             # [128, 4, 4, 256]
            "ow": np.ascontiguousarray(
                ow_r[HPC * hg: HPC * (hg + 1)].transpose(1, 0, 2)).astype(bf),
            "cos_f": cosT.astype(bf),
            "sin_f": sinT.astype(bf),
            "masks": masks,
        })
    return in_maps


def _postprocess(results):
    out = np.empty((B, S, HID), dtype=np.float32)
    for b in range(B):
        acc = results[4 * b]["out_t"].astype(np.float32)
        for c in GROUPS[b][1:]:
            acc = acc + results[c]["out_t"].astype(np.float32)
        out[b] = acc.T
    return out


def kernel(**inputs):
    key = (str(MM_DT),)
    if key not in _CACHE:
        _CACHE[key] = build_kernel(MM_DT)
    nc = _CACHE[key]
    in_maps = _prep_inputs(**inputs)
    r = run_bass_kernel_spmd(nc, in_maps, core_ids=list(range(N_CORES)))
    return _postprocess(r.results)


# revision 41
# speedup vs baseline: 1.0851x; 1.0797x over previous
"""DeepseekV2 MLA attention prefill kernel for 8 Trainium2 NeuronCores.

Sharding: 2-way data-parallel over batch x 4-way tensor-parallel over heads
(4 heads per core).  The raw q down-projection (+ rstd of its RMS norm) is
computed on an S/4 slice per core and exchanged with one in-group AllGather;
the RMS normalization is folded into the q up-projection output after the
gather.  The compressed-KV path is replicated at full S on every core and
computed while the gather is in flight.  Per-head up-projections, attention
and the output projection are computed locally; o_proj partial sums are
reduced on the host during unsharding.

Key scheduling/efficiency points (v4):
 - score matmuls run in fp8e4 DoubleRow: the two 128-deep k-subtiles are
   [k_nope | (k_pe ; zeros)], so one PE pass per 128x512 score block covers
   the full 192-dim contraction (rope included); q/k packs are built by the
   DVE/DMA on the side.  Everything else stays bf16 (fp8 there fails the
   2e-2 tolerance; scores measured 1.0e-2 in emulation).
 - the collective lives alone on the gpsimd queue; RMS rstd broadcasts are
   done by a K=1 PE matmul against a ones row so the kv-norm never blocks
   behind the 90us gather.
 - DMA priority at startup: only the q-down critical stream (xt_loc + wdq)
   is issued first; all other weights follow it on the same queue.
 - exp is evaluated over [128, 1024] pairs of score banks (halves ScalarE
   instruction overhead); attention context is evacuated raw and the
   softmax 1/sum is applied during a later DVE pass, so the single-bank
   ctx accumulator frees immediately at head boundaries.
 - PV and row-sum matmuls restrict their free dim on diagonal blocks.

Layouts: activations are feature-major ([D, S]); scores are computed
transposed ([s_k, s_q]) so PV needs no transposes.  RoPE uses host-side
permuted/sign-folded weight columns.  PSUM accumulation fp32 throughout.
"""
import sys
sys.path.insert(0, "/opt/trn_rl_repo")

import math
import numpy as np
import ml_dtypes

import concourse.bass as bass
import concourse.tile as tile
from concourse import bacc, mybir
from concourse.bass_utils import run_bass_kernel_spmd

# ---- problem constants (hardcoded; kernel.py must be self-contained) ----
B, S, HID, H = 2, 2048, 2048, 16
Q_LORA, KV_LORA = 1536, 512
D_NOPE, D_ROPE, D_V = 128, 64, 128
D_Q = D_NOPE + D_ROPE
EPS = 1e-6
ROPE_THETA = 10000.0
N_CORES = 8
HPC = 4                      # heads per core
GROUPS = [[0, 1, 2, 3], [4, 5, 6, 7]]

KC = HID // 128              # 16
QC = Q_LORA // 128           # 12
VC = KV_LORA // 128          # 4
NSK = S // 128               # 16 key blocks

F32 = mybir.dt.float32
BF16 = mybir.dt.bfloat16
F8 = mybir.dt.float8e4
MM_DT = BF16
DR = mybir.MatmulPerfMode.DoubleRow

SCALE = 1.0 / math.sqrt(D_Q)

_CACHE = {}


# ---------------------------------------------------------------- builder --
def build_kernel(mm_dt=MM_DT):
    s_loc = S // 4

    nc = bacc.Bacc("TRN2", target_bir_lowering=False, debug=False,
                   num_devices=N_CORES)

    xt = nc.dram_tensor("xt", [HID, S], mm_dt, kind="ExternalInput")
    xt_loc = nc.dram_tensor("xt_loc", [HID, s_loc], mm_dt, kind="ExternalInput")
    wdq = nc.dram_tensor("wdq", [HID, Q_LORA], mm_dt, kind="ExternalInput")
    wuq = nc.dram_tensor("wuq", [128, QC, HPC * 256], mm_dt, kind="ExternalInput")
    wkva = nc.dram_tensor("wkva", [128, KC, KV_LORA + 2 * D_ROPE], mm_dt,
                          kind="ExternalInput")
    wkvb = nc.dram_tensor("wkvb", [128, VC, HPC, 256], mm_dt, kind="ExternalInput")
    ow = nc.dram_tensor("ow", [D_V, HPC, HID], mm_dt, kind="ExternalInput")
    cos_f = nc.dram_tensor("cos_f", [D_ROPE, S], mm_dt, kind="ExternalInput")
    sin_f = nc.dram_tensor("sin_f", [D_ROPE, S], mm_dt, kind="ExternalInput")
    masks = nc.dram_tensor("masks", [128, 4, 512], mm_dt, kind="ExternalInput")
    out_t = nc.dram_tensor("out_t", [HID, S], mm_dt, kind="ExternalOutput")

    with tile.TileContext(nc) as tc:
        import contextlib
        ctx = contextlib.ExitStack()
        with ctx:
            persist = ctx.enter_context(tc.tile_pool(name="persist", bufs=1))
            wpool = ctx.enter_context(tc.tile_pool(name="wpool", bufs=3))
            spool = ctx.enter_context(tc.tile_pool(name="spool", bufs=2))
            xpool = ctx.enter_context(tc.tile_pool(name="xpool", bufs=3))
            # PSUM: ppool 2 + pscore 2x2banks + pctx 1 + psums 1 = 8 banks
            ppool = ctx.enter_context(tc.tile_pool(name="ppool", bufs=2, space="PSUM"))
            pscore = ctx.enter_context(tc.tile_pool(name="pscore", bufs=2, space="PSUM"))
            pctx = ctx.enter_context(tc.tile_pool(name="pctx", bufs=1, space="PSUM"))
            psums = ctx.enter_context(tc.tile_pool(name="psums", bufs=1, space="PSUM"))
            dram = ctx.enter_context(tc.tile_pool(name="dram", bufs=1, space="DRAM"))

            ones_sb = persist.tile([128, 1], mm_dt, tag="ones")
            nc.vector.memset(ones_sb, 1.0)
            onesr_sb = persist.tile([1, 128], mm_dt, tag="onesr")
            nc.vector.memset(onesr_sb, 1.0)
            eps_sb = persist.tile([1, 1], F32, tag="eps")
            nc.vector.memset(eps_sb, EPS)

            # fused fp8 key pack: [d(128), h, skt, {nope | rope}, s_k(128)];
            # rope rows 64-127 are zero so the q-side values there are inert
            kf_sb = persist.tile([128, HPC, NSK, 2, 128], F8, tag="kf")
            nc.vector.memset(kf_sb[64:128, :, :, 1, :], 0.0)
            # fp8 q packs: rows 64-127 of the rope subtile are never written,
            # and uninitialized fp8 bytes can decode as NaN (NaN*0=NaN in the
            # PE), so zero them once up front.
            qf_t = {}
            for h in range(HPC):
                qf_t[h] = persist.tile([128, 2, 512], F8, tag="qf_h%d" % h,
                                       name="qf%d" % h)
                nc.vector.memset(qf_t[h][64:128, 1, :], 0.0)

            # ---- q-down critical DMA stream first: xt_loc + wdq ----
            xtl_sb = persist.tile([128, KC, 512], mm_dt, tag="xtl")
            mgs = [list(range(g, g + 4)) for g in range(0, QC, 4)]
            wdq_t = {}
            for k in range(KC):
                nc.scalar.dma_start(out=xtl_sb[:, k, :],
                                    in_=xt_loc.ap()[k * 128:(k + 1) * 128, :])
                wt = wpool.tile([128, 512], mm_dt, tag="w_s1", bufs=6)
                nc.sync.dma_start(
                    out=wt, in_=wdq.ap()[k * 128:(k + 1) * 128, 0:512])
                wdq_t[(0, k)] = wt
            for gi in range(1, 3):
                for k in range(KC):
                    wt = wpool.tile([128, 512], mm_dt, tag="w_s1", bufs=6)
                    nc.sync.dma_start(
                        out=wt,
                        in_=wdq.ap()[k * 128:(k + 1) * 128,
                                     gi * 512:(gi + 1) * 512])
                    wdq_t[(gi, k)] = wt

            # gather buffers (DRAM): 12 raw q-down chunks + broadcast rstd
            GR = Q_LORA + 128
            g_in = dram.tile([GR, s_loc], mm_dt)
            g_out = dram.tile([4 * GR, 512], mm_dt)

            # ---- stage 1a: q down-proj (raw) + RMS stats on local slice ----
            ssq_q = psums.tile([1, 512], F32, tag="p_sum", name="ssq_q")
            for gi, mg in enumerate(mgs):
                a2 = [pscore.tile([128, 2, 512], F32, tag="p_sc2", name="acc2")
                      for _ in range(2)]
                accs = {m: a2[j // 2][:, j % 2] for j, m in enumerate(mg)}
                for k in range(KC):
                    wt = wdq_t.pop((gi, k))
                    for j, m in enumerate(mg):
                        nc.tensor.matmul(
                            accs[m], wt[:, j * 128:(j + 1) * 128], xtl_sb[:, k, :],
                            start=(k == 0), stop=(k == KC - 1))
                for m in mg:
                    sq = spool.tile([128, 512], mm_dt, tag="sq", bufs=1)
                    nc.scalar.activation(out=sq, in_=accs[m],
                                         func=mybir.ActivationFunctionType.Square)
                    nc.tensor.matmul(ssq_q, ones_sb, sq,
                                     start=(m == 0), stop=(m == QC - 1),
                                     skip_group_check=True)
                    r = spool.tile([128, 512], mm_dt, tag="qdout%d" % (m % 2), bufs=1)
                    nc.vector.tensor_copy(r, accs[m])
                    nc.sync.dma_start(out=g_in[m * 128:(m + 1) * 128, :], in_=r)
            sd = spool.tile([1, 512], F32, tag="sdn", bufs=1)
            nc.scalar.activation(out=sd, in_=ssq_q,
                                 func=mybir.ActivationFunctionType.Sqrt,
                                 bias=eps_sb, scale=1.0 / Q_LORA)
            rstd = spool.tile([1, 512], F32, tag="rstdn", bufs=1)
            nc.vector.reciprocal(rstd, sd)
            rstd_b = spool.tile([1, 512], mm_dt, tag="rstdb")
            nc.vector.tensor_copy(rstd_b, rstd)
            # partition-broadcast via K=1 matmul (gpsimd only has the gather)
            rbc_ps = ppool.tile([128, 512], F32, tag="p_a", name="rbc_q")
            nc.tensor.matmul(rbc_ps, onesr_sb, rstd_b, start=True, stop=True)
            rstd_bcb = spool.tile([128, 512], mm_dt, tag="rstd_bcb_q", bufs=1)
            nc.vector.tensor_copy(rstd_bcb, rbc_ps)
            nc.sync.dma_start(out=g_in[Q_LORA:GR, :], in_=rstd_bcb)

            # ---- stage 1b: AllGather within batch groups (gpsimd queue) ----
            nc.gpsimd.collective_compute(
                "AllGather", mybir.AluOpType.bypass,
                replica_groups=GROUPS,
                ins=[g_in.opt()], outs=[g_out.opt()])

            # remaining weights (sync queue, behind the q-down stream)
            wkva_sb = persist.tile([128, KC, KV_LORA + 2 * D_ROPE], mm_dt, tag="wkva")
            nc.sync.dma_start(out=wkva_sb, in_=wkva.ap())
            wkvb_sb = persist.tile([128, VC, HPC, 256], mm_dt, tag="wkvb")
            nc.sync.dma_start(out=wkvb_sb, in_=wkvb.ap())
            mask_sb = persist.tile([128, 4, 512], mm_dt, tag="masks")
            nc.sync.dma_start(out=mask_sb, in_=masks.ap())
            cosf_sb = persist.tile([D_ROPE, 4, 512], mm_dt, tag="cosf")
            sinf_sb = persist.tile([D_ROPE, 4, 512], mm_dt, tag="sinf")
            nc.sync.dma_start(out=cosf_sb,
                              in_=cos_f.ap().rearrange("d (c n) -> d c n", c=4))
            nc.sync.dma_start(out=sinf_sb,
                              in_=sin_f.ap().rearrange("d (c n) -> d c n", c=4))

            # ---- stage 1c (overlaps gather): compressed KV at full S ----
            ckv_sb = persist.tile([128, VC, 4, 512], mm_dt, tag="ckv")
            for nch in range(4):
                ssq = psums.tile([1, 512], F32, tag="p_sum", name="ssq_kv")
                a2 = [pscore.tile([128, 2, 512], F32, tag="p_sc2", name="acc2")
                      for _ in range(2)]
                accs = {m: a2[m // 2][:, m % 2] for m in range(4)}
                acc_r = ppool.tile([128, 512], F32, tag="p_a", name="acc_rope")
                accs[4] = acc_r
                for k in range(KC):
                    xtt = xpool.tile([128, 512], mm_dt, tag="xt_s", bufs=24)
                    nc.sync.dma_start(
                        out=xtt,
                        in_=xt.ap()[k * 128:(k + 1) * 128,
                                    nch * 512:(nch + 1) * 512])
                    for m in range(5):
                        nc.tensor.matmul(
                            accs[m], wkva_sb[:, k, m * 128:(m + 1) * 128], xtt,
                            start=(k == 0), stop=(k == KC - 1))
                raw = []
                for m in range(4):
                    sq = spool.tile([128, 512], mm_dt, tag="sq", bufs=1)
                    nc.scalar.activation(out=sq, in_=accs[m],
                                         func=mybir.ActivationFunctionType.Square)
                    nc.tensor.matmul(ssq, ones_sb, sq,
                                     start=(m == 0), stop=(m == 3),
                                     skip_group_check=True)
                    r = spool.tile([128, 512], mm_dt, tag="kvraw%d" % m, bufs=1)
                    nc.vector.tensor_copy(r, accs[m])
                    raw.append((m, r))
                # rope chunk [E(64) | R(64)] -> k_pe (fp8), fanned into kf
                t0 = spool.tile([D_ROPE, 512], mm_dt, tag="ropet0", bufs=1)
                t1 = spool.tile([D_ROPE, 512], mm_dt, tag="ropet1", bufs=1)
                nc.vector.tensor_tensor(t0, acc_r[0:D_ROPE, :],
                                        cosf_sb[:, nch, :], mybir.AluOpType.mult)
                nc.vector.tensor_tensor(t1, acc_r[D_ROPE:2 * D_ROPE, :],
                                        sinf_sb[:, nch, :], mybir.AluOpType.mult)
                pe8 = spool.tile([D_ROPE, 512], F8, tag="ropeo8")
                nc.vector.tensor_tensor(pe8, t0, t1, mybir.AluOpType.add)
                for h in range(HPC):
                    nc.sync.dma_start(
                        out=kf_sb[0:64, h, 4 * nch:4 * nch + 4, 1, :],
                        in_=pe8)
                sd2 = spool.tile([1, 512], F32, tag="sdn", bufs=1)
                nc.scalar.activation(out=sd2, in_=ssq,
                                     func=mybir.ActivationFunctionType.Sqrt,
                                     bias=eps_sb, scale=1.0 / KV_LORA)
                rstd2 = spool.tile([1, 512], F32, tag="rstdn", bufs=1)
                nc.vector.reciprocal(rstd2, sd2)
                rstd2_b = spool.tile([1, 512], mm_dt, tag="rstdb")
                nc.vector.tensor_copy(rstd2_b, rstd2)
                rbc2 = ppool.tile([128, 512], F32, tag="p_a", name="rbc_kv")
                nc.tensor.matmul(rbc2, onesr_sb, rstd2_b, start=True, stop=True)
                for m, r in raw:
                    nc.vector.tensor_tensor(ckv_sb[:, m, nch, :], r, rbc2,
                                            mybir.AluOpType.mult)

            # ---- stage 2a: decompress KV (full S, local heads) ----
            for h in range(HPC):
                for skc in range(4):
                    acc = ppool.tile([128, 512], F32, tag="p_a", name="acc_kn")
                    for k in range(VC):
                        nc.tensor.matmul(acc, wkvb_sb[:, k, h, 0:128],
                                         ckv_sb[:, k, skc, :],
                                         start=(k == 0), stop=(k == VC - 1))
                    nc.vector.tensor_copy(
                        kf_sb[:, h, 4 * skc:4 * skc + 4, 0, :], acc)

            v_sb = persist.tile([128, NSK, HPC * D_V], mm_dt, tag="v")
            for skt in range(NSK):
                acc = ppool.tile([128, 512], F32, tag="p_a", name="acc_v")
                for k in range(VC):
                    nc.tensor.matmul(
                        acc,
                        ckv_sb[:, k, skt // 4, (skt % 4) * 128:(skt % 4) * 128 + 128],
                        wkvb_sb[:, k, :, 128:256],
                        start=(k == 0), stop=(k == VC - 1))
                nc.vector.tensor_copy(v_sb[:, skt, :], acc)

            # ---- stage 2b/2c/2d: per-seq-chunk q up-proj, attention, o ----
            for sqc in range(4):
                qnorm_t = [spool.tile([128, 512], mm_dt, tag="qn_stream%d" % (k % 4),
                                      name="qnorm_t", bufs=3) for k in range(QC)]
                for k in range(QC):
                    nc.sync.dma_start(
                        out=qnorm_t[k],
                        in_=g_out[sqc * GR + k * 128: sqc * GR + (k + 1) * 128, :])
                rstd_t = spool.tile([128, 512], mm_dt, tag="rstd_t", bufs=1)
                nc.sync.dma_start(out=rstd_t,
                                  in_=g_out[sqc * GR + Q_LORA: (sqc + 1) * GR, :])
                for h in range(HPC):
                    wts = []
                    for k in range(QC):
                        wt = wpool.tile([128, 256], mm_dt, tag="w_uq")
                        nc.sync.dma_start(
                            out=wt, in_=wuq.ap()[:, k, h * 256:(h + 1) * 256])
                        wts.append(wt)
                    acc2 = pscore.tile([128, 2, 512], F32, tag="p_sc2", name="acc_qup")
                    for k in range(QC):
                        for j in range(2):
                            nc.tensor.matmul(
                                acc2[:, j], wts[k][:, j * 128:(j + 1) * 128],
                                qnorm_t[k],
                                start=(k == 0), stop=(k == QC - 1))
                    # fp8 q pack [nope | rope]; RMS rstd folded in here
                    qf = qf_t[h]
                    nc.vector.tensor_tensor(qf[:, 0, :], acc2[:, 0], rstd_t,
                                            mybir.AluOpType.mult)
                    t0 = spool.tile([D_ROPE, 512], mm_dt, tag="ropet0", bufs=1)
                    t1 = spool.tile([D_ROPE, 512], mm_dt, tag="ropet1", bufs=1)
                    nc.vector.tensor_tensor(t0, acc2[0:D_ROPE, 1], cosf_sb[:, sqc, :],
                                            mybir.AluOpType.mult)
                    nc.vector.tensor_tensor(t1, acc2[D_ROPE:2 * D_ROPE, 1],
                                            sinf_sb[:, sqc, :], mybir.AluOpType.mult)
                    t2 = spool.tile([D_ROPE, 512], mm_dt, tag="ropeo")
                    nc.vector.tensor_tensor(t2, t0, t1, mybir.AluOpType.add)
                    nc.vector.tensor_tensor(qf[0:64, 1, :], t2, rstd_t[0:64, :],
                                            mybir.AluOpType.mult)
                    qf_t[h] = qf

                n_skt = 4 * (sqc + 1)
                ctx_sb = spool.tile([D_V, HPC, 512], mm_dt, tag="ctx", bufs=2)
                for h in range(HPC):
                    sum_acc = psums.tile([1, 512], F32, tag="p_sum", name="sum_acc")
                    ctx_acc = pctx.tile([D_V, 512], F32, tag="p_ctx")

                    def drain(pex2, pskp):
                        for half in range(2):
                            skt = 2 * pskp + half
                            pd = skt - 4 * sqc
                            c0 = 128 * pd if pd > 0 else 0
                            pex = pex2[:, half]
                            nc.tensor.matmul(sum_acc[:, c0:], ones_sb,
                                             pex[:, c0:],
                                             start=(skt == 0),
                                             stop=(skt == n_skt - 1),
                                             skip_group_check=True)
                            nc.tensor.matmul(ctx_acc[:, c0:],
                                             v_sb[:, skt, h * D_V:(h + 1) * D_V],
                                             pex[:, c0:],
                                             start=(skt == 0),
                                             stop=(skt == n_skt - 1),
                                             skip_group_check=True)

                    pending = None   # software pipeline: exp pair awaiting PV
                    for skp in range(n_skt // 2):
                        sc2 = pscore.tile([128, 2, 512], F32, tag="p_sc2",
                                          name="sc2")
                        for half in range(2):
                            skt = 2 * skp + half
                            nc.tensor.matmul(
                                sc2[:, half], kf_sb[:, h, skt], qf_t[h],
                                start=True, stop=True, perf_mode=DR,
                                skip_group_check=True)
                        ex2 = spool.tile([128, 2, 512], mm_dt,
                                         tag="exp%d" % (skp % 2), bufs=2)
                        nc.scalar.activation(out=ex2, in_=sc2,
                                             func=mybir.ActivationFunctionType.Exp,
                                             scale=SCALE)
                        d0 = 2 * skp - 4 * sqc
                        if d0 >= 0:
                            nc.vector.tensor_tensor(ex2, ex2,
                                                    mask_sb[:, d0:d0 + 2, :],
                                                    mybir.AluOpType.mult)
                        if pending is not None:
                            drain(*pending)
                        pending = (ex2, skp)
                    drain(*pending)
                    # raw evacuation frees the single ctx bank immediately
                    ctxr = spool.tile([D_V, 512], mm_dt, tag="ctxr%d" % (h % 2), bufs=1)
                    nc.vector.tensor_copy(ctxr, ctx_acc)
                    # 1/sum via exp(-ln(sum)) on ScalarE
                    ls = spool.tile([1, 512], F32, tag="lsum", bufs=1)
                    nc.scalar.activation(out=ls, in_=sum_acc,
                                         func=mybir.ActivationFunctionType.Ln)
                    rc = spool.tile([1, 512], F32, tag="recip1", bufs=1)
                    nc.scalar.activation(out=rc, in_=ls, scale=-1.0,
                                         func=mybir.ActivationFunctionType.Exp)
                    rb = spool.tile([128, 512], F32, tag="recip_bc", bufs=1)
                    nc.gpsimd.partition_broadcast(rb, rc)
                    nc.vector.tensor_tensor(ctx_sb[:, h, :], ctxr, rb,
                                            mybir.AluOpType.mult)

                # ---- output projection for this seq chunk ----
                for hidc in range(HID // 128):
                    owt = wpool.tile([D_V, HPC, 128], mm_dt, tag="w_o")
                    nc.sync.dma_start(
                        out=owt, in_=ow.ap()[:, :, hidc * 128:(hidc + 1) * 128])
                    acc = ppool.tile([128, 512], F32, tag="p_a", name="acc_o")
                    for h in range(HPC):
                        nc.tensor.matmul(acc, owt[:, h, :], ctx_sb[:, h, :],
                                         start=(h == 0), stop=(h == HPC - 1))
                    o = spool.tile([128, 512], mm_dt, tag="oout", bufs=1)
                    nc.vector.tensor_copy(o, acc)
                    nc.gpsimd.dma_start(
                        out=out_t.ap()[hidc * 128:(hidc + 1) * 128,
                                       sqc * 512:(sqc + 1) * 512],
                        in_=o)

    nc.compile()
    return nc


# ------------------------------------------------------------- host side --
def _rope_tables():
    inv_freq = 1.0 / (ROPE_THETA ** (np.arange(0, D_ROPE, 2, dtype=np.float64) / D_ROPE))
    t = np.arange(S, dtype=np.float64)
    freqs = np.outer(t, inv_freq)                    # [S, 32]
    emb = np.concatenate([freqs, freqs], axis=-1)    # [S, 64]
    return (np.cos(emb).astype(np.float32).T.copy(),
            np.sin(emb).astype(np.float32).T.copy())  # [64, S]


_E_PERM = np.concatenate([np.arange(0, D_ROPE, 2), np.arange(1, D_ROPE, 2)])


def _rope_expand(Wpe):
    """[n, 64] rope weight cols -> [n, 128]: [even/odd-reordered | rot-half signed]."""
    Y = Wpe[:, _E_PERM]
    R = np.concatenate([-Y[:, D_ROPE // 2:], Y[:, :D_ROPE // 2]], axis=1)
    return np.concatenate([Y, R], axis=1)


def _chunk_rows(a, p=128):
    """[R, ...] -> [p, R//p, ...] grouping rows into chunks of p."""
    R, Cs = a.shape[0], a.shape[1:]
    return np.ascontiguousarray(a.reshape(R // p, p, *Cs).transpose(
        1, 0, *range(2, a.ndim + 1)))


def _prep_inputs(hidden_states, w_dq, q_a_ln_w, w_uq, kv_a_w, kv_a_ln_w, kv_b_w, o_w):
    bf = ml_dtypes.bfloat16
    s_loc = S // 4
    cosT, sinT = _rope_tables()

    wuq_eff = (np.asarray(q_a_ln_w)[:, None] * np.asarray(w_uq)).reshape(Q_LORA, H, D_Q)
    head_blocks = []
    for h in range(H):
        head_blocks.append(np.concatenate(
            [wuq_eff[:, h, :D_NOPE], _rope_expand(wuq_eff[:, h, D_NOPE:])], axis=1))
    wuq_x = np.stack(head_blocks, axis=1)            # [1536, 16, 256]

    kv_a = np.asarray(kv_a_w)
    wkva_x = np.concatenate([kv_a[:, :KV_LORA], _rope_expand(kv_a[:, KV_LORA:])],
                            axis=1).astype(bf)       # [2048, 640]
    wkva_p = _chunk_rows(wkva_x)                     # [128, 16, 640]
    wkvb_eff = (np.asarray(kv_a_ln_w)[:, None] * np.asarray(kv_b_w)).reshape(KV_LORA, H, 256)
    ow_r = np.asarray(o_w).reshape(H, D_V, HID)

    c_idx = np.arange(512)[None, :]
    r_idx = np.arange(128)[:, None]
    masks = np.stack([(c_idx >= 128 * dd + r_idx) for dd in range(4)],
                     axis=1).astype(bf)              # [128, 4, 512]

    wdq_b = np.asarray(w_dq).astype(bf)
    hs = np.asarray(hidden_states)

    in_maps = []
    for c in range(N_CORES):
        b, hg = c // 4, c % 4
        s0 = 512 * hg
        xt_full = np.ascontiguousarray(hs[b].T).astype(bf)
        wuq_c = wuq_x[:, HPC * hg: HPC * (hg + 1), :].reshape(
            Q_LORA, HPC * 256).astype(bf)
        wkvb_c = wkvb_eff[:, HPC * hg: HPC * (hg + 1), :].astype(bf)
        in_maps.append({
            "xt": xt_full,
            "xt_loc": np.ascontiguousarray(xt_full[:, s0:s0 + s_loc]),
            "wdq": wdq_b,
            "wuq": _chunk_rows(wuq_c),               # [128, 12, 1024]
            "wkva": wkva_p,
            "wkvb": _chunk_rows(wkvb_c),             # [128, 4, 4, 256]
            "ow": np.ascontiguousarray(
                ow_r[HPC * hg: HPC * (hg + 1)].transpose(1, 0, 2)).astype(bf),
            "cos_f": cosT.astype(bf),
            "sin_f": sinT.astype(bf),
            "masks": masks,
        })
    return in_maps


def _postprocess(results):
    out = np.empty((B, S, HID), dtype=np.float32)
    for b in range(B):
        acc = results[4 * b]["out_t"].astype(np.float32)
        for c in GROUPS[b][1:]:
            acc = acc + results[c]["out_t"].astype(np.float32)
        out[b] = acc.T
    return out


def kernel(**inputs):
    key = (str(MM_DT),)
    if key not in _CACHE:
        _CACHE[key] = build_kernel(MM_DT)
    nc = _CACHE[key]
    in_maps = _prep_inputs(**inputs)
    r = run_bass_kernel_spmd(nc, in_maps, core_ids=list(range(N_CORES)))
    return _postprocess(r.results)


# revision 43
# speedup vs baseline: 1.0874x; 1.0021x over previous
"""DeepseekV2 MLA attention prefill kernel for 8 Trainium2 NeuronCores.

Sharding: 2-way data-parallel over batch x 4-way tensor-parallel over heads
(4 heads per core).  The raw q down-projection (+ rstd of its RMS norm) is
computed on an S/4 slice per core and exchanged with one in-group AllGather;
the RMS normalization is folded into the q up-projection output after the
gather.  The compressed-KV path is replicated at full S on every core and
computed while the gather is in flight.  Per-head up-projections, attention
and the output projection are computed locally; o_proj partial sums are
reduced on the host during unsharding.

Key scheduling/efficiency points (v4):
 - score matmuls run in fp8e4 DoubleRow: the two 128-deep k-subtiles are
   [k_nope | (k_pe ; zeros)], so one PE pass per 128x512 score block covers
   the full 192-dim contraction (rope included); q/k packs are built by the
   DVE/DMA on the side.  Everything else stays bf16 (fp8 there fails the
   2e-2 tolerance; scores measured 1.0e-2 in emulation).
 - the collective lives alone on the gpsimd queue; RMS rstd broadcasts are
   done by a K=1 PE matmul against a ones row so the kv-norm never blocks
   behind the 90us gather.
 - DMA priority at startup: only the q-down critical stream (xt_loc + wdq)
   is issued first; all other weights follow it on the same queue.
 - exp is evaluated over [128, 1024] pairs of score banks (halves ScalarE
   instruction overhead); attention context is evacuated raw and the
   softmax 1/sum is applied during a later DVE pass, so the single-bank
   ctx accumulator frees immediately at head boundaries.
 - PV and row-sum matmuls restrict their free dim on diagonal blocks.

Layouts: activations are feature-major ([D, S]); scores are computed
transposed ([s_k, s_q]) so PV needs no transposes.  RoPE uses host-side
permuted/sign-folded weight columns.  PSUM accumulation fp32 throughout.
"""
import sys
sys.path.insert(0, "/opt/trn_rl_repo")

import math
import numpy as np
import ml_dtypes

import concourse.bass as bass
import concourse.tile as tile
from concourse import bacc, mybir
from concourse.bass_utils import run_bass_kernel_spmd

# ---- problem constants (hardcoded; kernel.py must be self-contained) ----
B, S, HID, H = 2, 2048, 2048, 16
Q_LORA, KV_LORA = 1536, 512
D_NOPE, D_ROPE, D_V = 128, 64, 128
D_Q = D_NOPE + D_ROPE
EPS = 1e-6
ROPE_THETA = 10000.0
N_CORES = 8
HPC = 4                      # heads per core
GROUPS = [[0, 1, 2, 3], [4, 5, 6, 7]]

KC = HID // 128              # 16
QC = Q_LORA // 128           # 12
VC = KV_LORA // 128          # 4
NSK = S // 128               # 16 key blocks

F32 = mybir.dt.float32
BF16 = mybir.dt.bfloat16
F8 = mybir.dt.float8e4
MM_DT = BF16
DR = mybir.MatmulPerfMode.DoubleRow

SCALE = 1.0 / math.sqrt(D_Q)

_CACHE = {}


# ---------------------------------------------------------------- builder --
def build_kernel(mm_dt=MM_DT):
    s_loc = S // 4

    nc = bacc.Bacc("TRN2", target_bir_lowering=False, debug=False,
                   num_devices=N_CORES)

    xt = nc.dram_tensor("xt", [HID, S], mm_dt, kind="ExternalInput")
    xt_loc = nc.dram_tensor("xt_loc", [HID, s_loc], mm_dt, kind="ExternalInput")
    wdq = nc.dram_tensor("wdq", [HID, Q_LORA], mm_dt, kind="ExternalInput")
    wuq = nc.dram_tensor("wuq", [128, QC, HPC * 256], mm_dt, kind="ExternalInput")
    wkva = nc.dram_tensor("wkva", [128, KC, KV_LORA + 2 * D_ROPE], mm_dt,
                          kind="ExternalInput")
    wkvb = nc.dram_tensor("wkvb", [128, VC, HPC, 256], mm_dt, kind="ExternalInput")
    ow = nc.dram_tensor("ow", [D_V, HPC, HID], mm_dt, kind="ExternalInput")
    cos_f = nc.dram_tensor("cos_f", [D_ROPE, S], mm_dt, kind="ExternalInput")
    sin_f = nc.dram_tensor("sin_f", [D_ROPE, S], mm_dt, kind="ExternalInput")
    masks = nc.dram_tensor("masks", [128, 4, 512], mm_dt, kind="ExternalInput")
    out_t = nc.dram_tensor("out_t", [HID, S], mm_dt, kind="ExternalOutput")

    with tile.TileContext(nc) as tc:
        import contextlib
        ctx = contextlib.ExitStack()
        with ctx:
            persist = ctx.enter_context(tc.tile_pool(name="persist", bufs=1))
            wpool = ctx.enter_context(tc.tile_pool(name="wpool", bufs=3))
            spool = ctx.enter_context(tc.tile_pool(name="spool", bufs=2))
            xpool = ctx.enter_context(tc.tile_pool(name="xpool", bufs=3))
            # PSUM: ppool 2 + pscore 2x2banks + pctx 1 + psums 1 = 8 banks
            ppool = ctx.enter_context(tc.tile_pool(name="ppool", bufs=2, space="PSUM"))
            pscore = ctx.enter_context(tc.tile_pool(name="pscore", bufs=2, space="PSUM"))
            pctx = ctx.enter_context(tc.tile_pool(name="pctx", bufs=1, space="PSUM"))
            psums = ctx.enter_context(tc.tile_pool(name="psums", bufs=1, space="PSUM"))
            dram = ctx.enter_context(tc.tile_pool(name="dram", bufs=1, space="DRAM"))

            ones_sb = persist.tile([128, 1], mm_dt, tag="ones")
            nc.vector.memset(ones_sb, 1.0)
            onesr_sb = persist.tile([1, 128], mm_dt, tag="onesr")
            nc.vector.memset(onesr_sb, 1.0)
            eps_sb = persist.tile([1, 1], F32, tag="eps")
            nc.vector.memset(eps_sb, EPS)

            # fused fp8 key pack: [d(128), h, skt, {nope | rope}, s_k(128)];
            # rope rows 64-127 are zero so the q-side values there are inert
            kf_sb = persist.tile([128, HPC, NSK, 2, 128], F8, tag="kf")
            nc.vector.memset(kf_sb[64:128, :, :, 1, :], 0.0)
            # fp8 q packs: rows 64-127 of the rope subtile are never written,
            # and uninitialized fp8 bytes can decode as NaN (NaN*0=NaN in the
            # PE), so zero them once up front.
            qf_t = {}
            for h in range(HPC):
                qf_t[h] = persist.tile([128, 2, 512], F8, tag="qf_h%d" % h,
                                       name="qf%d" % h)
                nc.vector.memset(qf_t[h][64:128, 1, :], 0.0)

            # ---- q-down critical DMA stream first: xt_loc + wdq ----
            xtl_sb = persist.tile([128, KC, 512], mm_dt, tag="xtl")
            mgs = [list(range(g, g + 4)) for g in range(0, QC, 4)]
            wdq_t = {}
            for k in range(KC):
                nc.scalar.dma_start(out=xtl_sb[:, k, :],
                                    in_=xt_loc.ap()[k * 128:(k + 1) * 128, :])
                wt = wpool.tile([128, 512], mm_dt, tag="w_s1", bufs=6)
                nc.sync.dma_start(
                    out=wt, in_=wdq.ap()[k * 128:(k + 1) * 128, 0:512])
                wdq_t[(0, k)] = wt
            for gi in range(1, 3):
                for k in range(KC):
                    wt = wpool.tile([128, 512], mm_dt, tag="w_s1", bufs=6)
                    nc.sync.dma_start(
                        out=wt,
                        in_=wdq.ap()[k * 128:(k + 1) * 128,
                                     gi * 512:(gi + 1) * 512])
                    wdq_t[(gi, k)] = wt

            # gather buffers (DRAM), 2-chunk: A = raw rows 0-1023
            # (mg0+mg1, triggered early), B = rows 1024-1535 + rstd (640)
            g_inA = dram.tile([1024, s_loc], mm_dt)
            g_inB = dram.tile([640, s_loc], mm_dt)
            g_outA = dram.tile([4 * 1024, 512], mm_dt)
            g_outB = dram.tile([4 * 640, 512], mm_dt)

            # ---- stage 1a: q down-proj (raw) + RMS stats on local slice ----
            ssq_q = psums.tile([1, 512], F32, tag="p_sum", name="ssq_q")
            for gi, mg in enumerate(mgs):
                a2 = [pscore.tile([128, 2, 512], F32, tag="p_sc2", name="acc2")
                      for _ in range(2)]
                accs = {m: a2[j // 2][:, j % 2] for j, m in enumerate(mg)}
                for k in range(KC):
                    wt = wdq_t.pop((gi, k))
                    for j, m in enumerate(mg):
                        nc.tensor.matmul(
                            accs[m], wt[:, j * 128:(j + 1) * 128], xtl_sb[:, k, :],
                            start=(k == 0), stop=(k == KC - 1))
                for m in mg:
                    sq = spool.tile([128, 512], mm_dt, tag="sq", bufs=1)
                    nc.scalar.activation(out=sq, in_=accs[m],
                                         func=mybir.ActivationFunctionType.Square)
                    nc.tensor.matmul(ssq_q, ones_sb, sq,
                                     start=(m == 0), stop=(m == QC - 1),
                                     skip_group_check=True)
                    r = spool.tile([128, 512], mm_dt, tag="qdout%d" % (m % 2), bufs=1)
                    nc.vector.tensor_copy(r, accs[m])
                    if gi < 2:
                        nc.sync.dma_start(
                            out=g_inA[(m % 8) * 128:(m % 8) * 128 + 128, :], in_=r)
                    else:
                        nc.sync.dma_start(
                            out=g_inB[(m % 4) * 128:(m % 4) * 128 + 128, :], in_=r)
                if gi == 1:
                    nc.gpsimd.collective_compute(
                        "AllGather", mybir.AluOpType.bypass,
                        replica_groups=GROUPS,
                        ins=[g_inA.opt()], outs=[g_outA.opt()])
            sd = spool.tile([1, 512], F32, tag="sdn", bufs=1)
            nc.scalar.activation(out=sd, in_=ssq_q,
                                 func=mybir.ActivationFunctionType.Sqrt,
                                 bias=eps_sb, scale=1.0 / Q_LORA)
            rstd = spool.tile([1, 512], F32, tag="rstdn", bufs=1)
            nc.vector.reciprocal(rstd, sd)
            rstd_b = spool.tile([1, 512], mm_dt, tag="rstdb")
            nc.vector.tensor_copy(rstd_b, rstd)
            # partition-broadcast via K=1 matmul (gpsimd only has the gather)
            rbc_ps = ppool.tile([128, 512], F32, tag="p_a", name="rbc_q")
            nc.tensor.matmul(rbc_ps, onesr_sb, rstd_b, start=True, stop=True)
            rstd_bcb = spool.tile([128, 512], mm_dt, tag="rstd_bcb_q", bufs=1)
            nc.vector.tensor_copy(rstd_bcb, rbc_ps)
            nc.sync.dma_start(out=g_inB[512:640, :], in_=rstd_bcb)

            # ---- stage 1b part B: rows 1024-1535 + rstd (gpsimd queue) ----
            nc.gpsimd.collective_compute(
                "AllGather", mybir.AluOpType.bypass,
                replica_groups=GROUPS,
                ins=[g_inB.opt()], outs=[g_outB.opt()])

            # remaining weights (sync queue, behind the q-down stream)
            wkva_sb = persist.tile([128, KC, KV_LORA + 2 * D_ROPE], mm_dt, tag="wkva")
            nc.sync.dma_start(out=wkva_sb, in_=wkva.ap())
            wkvb_sb = persist.tile([128, VC, HPC, 256], mm_dt, tag="wkvb")
            nc.sync.dma_start(out=wkvb_sb, in_=wkvb.ap())
            mask_sb = persist.tile([128, 4, 512], mm_dt, tag="masks")
            nc.sync.dma_start(out=mask_sb, in_=masks.ap())
            cosf_sb = persist.tile([D_ROPE, 4, 512], mm_dt, tag="cosf")
            sinf_sb = persist.tile([D_ROPE, 4, 512], mm_dt, tag="sinf")
            nc.sync.dma_start(out=cosf_sb,
                              in_=cos_f.ap().rearrange("d (c n) -> d c n", c=4))
            nc.sync.dma_start(out=sinf_sb,
                              in_=sin_f.ap().rearrange("d (c n) -> d c n", c=4))

            # ---- stage 1c (overlaps gather): compressed KV at full S ----
            ckv_sb = persist.tile([128, VC, 4, 512], mm_dt, tag="ckv")
            for nch in range(4):
                ssq = psums.tile([1, 512], F32, tag="p_sum", name="ssq_kv")
                a2 = [pscore.tile([128, 2, 512], F32, tag="p_sc2", name="acc2")
                      for _ in range(2)]
                accs = {m: a2[m // 2][:, m % 2] for m in range(4)}
                acc_r = ppool.tile([128, 512], F32, tag="p_a", name="acc_rope")
                accs[4] = acc_r
                for k in range(KC):
                    xtt = xpool.tile([128, 512], mm_dt, tag="xt_s", bufs=24)
                    nc.sync.dma_start(
                        out=xtt,
                        in_=xt.ap()[k * 128:(k + 1) * 128,
                                    nch * 512:(nch + 1) * 512])
                    for m in range(5):
                        nc.tensor.matmul(
                            accs[m], wkva_sb[:, k, m * 128:(m + 1) * 128], xtt,
                            start=(k == 0), stop=(k == KC - 1))
                raw = []
                for m in range(4):
                    sq = spool.tile([128, 512], mm_dt, tag="sq", bufs=1)
                    nc.scalar.activation(out=sq, in_=accs[m],
                                         func=mybir.ActivationFunctionType.Square)
                    nc.tensor.matmul(ssq, ones_sb, sq,
                                     start=(m == 0), stop=(m == 3),
                                     skip_group_check=True)
                    r = spool.tile([128, 512], mm_dt, tag="kvraw%d" % m, bufs=1)
                    nc.vector.tensor_copy(r, accs[m])
                    raw.append((m, r))
                # rope chunk [E(64) | R(64)] -> k_pe (fp8), fanned into kf
                t0 = spool.tile([D_ROPE, 512], mm_dt, tag="ropet0", bufs=1)
                t1 = spool.tile([D_ROPE, 512], mm_dt, tag="ropet1", bufs=1)
                nc.vector.tensor_tensor(t0, acc_r[0:D_ROPE, :],
                                        cosf_sb[:, nch, :], mybir.AluOpType.mult)
                nc.vector.tensor_tensor(t1, acc_r[D_ROPE:2 * D_ROPE, :],
                                        sinf_sb[:, nch, :], mybir.AluOpType.mult)
                pe8 = spool.tile([D_ROPE, 512], F8, tag="ropeo8")
                nc.vector.tensor_tensor(pe8, t0, t1, mybir.AluOpType.add)
                for h in range(HPC):
                    nc.sync.dma_start(
                        out=kf_sb[0:64, h, 4 * nch:4 * nch + 4, 1, :],
                        in_=pe8)
                sd2 = spool.tile([1, 512], F32, tag="sdn", bufs=1)
                nc.scalar.activation(out=sd2, in_=ssq,
                                     func=mybir.ActivationFunctionType.Sqrt,
                                     bias=eps_sb, scale=1.0 / KV_LORA)
                rstd2 = spool.tile([1, 512], F32, tag="rstdn", bufs=1)
                nc.vector.reciprocal(rstd2, sd2)
                rstd2_b = spool.tile([1, 512], mm_dt, tag="rstdb")
                nc.vector.tensor_copy(rstd2_b, rstd2)
                rbc2 = ppool.tile([128, 512], F32, tag="p_a", name="rbc_kv")
                nc.tensor.matmul(rbc2, onesr_sb, rstd2_b, start=True, stop=True)
                for m, r in raw:
                    nc.vector.tensor_tensor(ckv_sb[:, m, nch, :], r, rbc2,
                                            mybir.AluOpType.mult)

            # ---- stage 2a: decompress KV (full S, local heads) ----
            for h in range(HPC):
                for skc in range(4):
                    acc = ppool.tile([128, 512], F32, tag="p_a", name="acc_kn")
                    for k in range(VC):
                        nc.tensor.matmul(acc, wkvb_sb[:, k, h, 0:128],
                                         ckv_sb[:, k, skc, :],
                                         start=(k == 0), stop=(k == VC - 1))
                    nc.vector.tensor_copy(
                        kf_sb[:, h, 4 * skc:4 * skc + 4, 0, :], acc)

            v_sb = persist.tile([128, NSK, HPC * D_V], mm_dt, tag="v")
            for skt in range(NSK):
                acc = ppool.tile([128, 512], F32, tag="p_a", name="acc_v")
                for k in range(VC):
                    nc.tensor.matmul(
                        acc,
                        ckv_sb[:, k, skt // 4, (skt % 4) * 128:(skt % 4) * 128 + 128],
                        wkvb_sb[:, k, :, 128:256],
                        start=(k == 0), stop=(k == VC - 1))
                nc.vector.tensor_copy(v_sb[:, skt, :], acc)

            # ---- stage 2b/2c/2d: per-seq-chunk q up-proj, attention, o ----
            for sqc in range(4):
                qnorm_t = [spool.tile([128, 512], mm_dt, tag="qn_stream%d" % (k % 4),
                                      name="qnorm_t", bufs=3) for k in range(QC)]
                for k in range(QC):
                    if k < 8:
                        gsrc = g_outA[sqc * 1024 + k * 128:
                                      sqc * 1024 + (k + 1) * 128, :]
                    else:
                        gsrc = g_outB[sqc * 640 + (k - 8) * 128:
                                      sqc * 640 + (k - 7) * 128, :]
                    nc.sync.dma_start(out=qnorm_t[k], in_=gsrc)
                rstd_t = spool.tile([128, 512], mm_dt, tag="rstd_t", bufs=1)
                nc.sync.dma_start(out=rstd_t,
                                  in_=g_outB[sqc * 640 + 512: sqc * 640 + 640, :])
                for h in range(HPC):
                    wts = []
                    for k in range(QC):
                        wt = wpool.tile([128, 256], mm_dt, tag="w_uq")
                        nc.sync.dma_start(
                            out=wt, in_=wuq.ap()[:, k, h * 256:(h + 1) * 256])
                        wts.append(wt)
                    acc2 = pscore.tile([128, 2, 512], F32, tag="p_sc2", name="acc_qup")
                    for k in range(QC):
                        for j in range(2):
                            nc.tensor.matmul(
                                acc2[:, j], wts[k][:, j * 128:(j + 1) * 128],
                                qnorm_t[k],
                                start=(k == 0), stop=(k == QC - 1))
                    # fp8 q pack [nope | rope]; RMS rstd folded in here
                    qf = qf_t[h]
                    nc.vector.tensor_tensor(qf[:, 0, :], acc2[:, 0], rstd_t,
                                            mybir.AluOpType.mult)
                    t0 = spool.tile([D_ROPE, 512], mm_dt, tag="ropet0", bufs=1)
                    t1 = spool.tile([D_ROPE, 512], mm_dt, tag="ropet1", bufs=1)
                    nc.vector.tensor_tensor(t0, acc2[0:D_ROPE, 1], cosf_sb[:, sqc, :],
                                            mybir.AluOpType.mult)
                    nc.vector.tensor_tensor(t1, acc2[D_ROPE:2 * D_ROPE, 1],
                                            sinf_sb[:, sqc, :], mybir.AluOpType.mult)
                    t2 = spool.tile([D_ROPE, 512], mm_dt, tag="ropeo")
                    nc.vector.tensor_tensor(t2, t0, t1, mybir.AluOpType.add)
                    nc.vector.tensor_tensor(qf[0:64, 1, :], t2, rstd_t[0:64, :],
                                            mybir.AluOpType.mult)
                    qf_t[h] = qf

                n_skt = 4 * (sqc + 1)
                ctx_sb = spool.tile([D_V, HPC, 512], mm_dt, tag="ctx", bufs=2)
                for h in range(HPC):
                    sum_acc = psums.tile([1, 512], F32, tag="p_sum", name="sum_acc")
                    ctx_acc = pctx.tile([D_V, 512], F32, tag="p_ctx")

                    def drain(pex2, pskp):
                        for half in range(2):
                            skt = 2 * pskp + half
                            pd = skt - 4 * sqc
                            c0 = 128 * pd if pd > 0 else 0
                            pex = pex2[:, half]
                            nc.tensor.matmul(sum_acc[:, c0:], ones_sb,
                                             pex[:, c0:],
                                             start=(skt == 0),
                                             stop=(skt == n_skt - 1),
                                             skip_group_check=True)
                            nc.tensor.matmul(ctx_acc[:, c0:],
                                             v_sb[:, skt, h * D_V:(h + 1) * D_V],
                                             pex[:, c0:],
                                             start=(skt == 0),
                                             stop=(skt == n_skt - 1),
                                             skip_group_check=True)

                    pending = None   # software pipeline: exp pair awaiting PV
                    for skp in range(n_skt // 2):
                        sc2 = pscore.tile([128, 2, 512], F32, tag="p_sc2",
                                          name="sc2")
                        for half in range(2):
                            skt = 2 * skp + half
                            nc.tensor.matmul(
                                sc2[:, half], kf_sb[:, h, skt], qf_t[h],
                                start=True, stop=True, perf_mode=DR,
                                skip_group_check=True)
                        ex2 = spool.tile([128, 2, 512], mm_dt,
                                         tag="exp%d" % (skp % 2), bufs=2)
                        nc.scalar.activation(out=ex2, in_=sc2,
                                             func=mybir.ActivationFunctionType.Exp,
                                             scale=SCALE)
                        d0 = 2 * skp - 4 * sqc
                        if d0 >= 0:
                            nc.vector.tensor_tensor(ex2, ex2,
                                                    mask_sb[:, d0:d0 + 2, :],
                                                    mybir.AluOpType.mult)
                        if pending is not None:
                            drain(*pending)
                        pending = (ex2, skp)
                    drain(*pending)
                    # raw evacuation frees the single ctx bank immediately
                    ctxr = spool.tile([D_V, 512], mm_dt, tag="ctxr%d" % (h % 2), bufs=1)
                    nc.vector.tensor_copy(ctxr, ctx_acc)
                    # 1/sum via exp(-ln(sum)) on ScalarE
                    ls = spool.tile([1, 512], F32, tag="lsum", bufs=1)
                    nc.scalar.activation(out=ls, in_=sum_acc,
                                         func=mybir.ActivationFunctionType.Ln)
                    rc = spool.tile([1, 512], F32, tag="recip1", bufs=1)
                    nc.scalar.activation(out=rc, in_=ls, scale=-1.0,
                                         func=mybir.ActivationFunctionType.Exp)
                    rb = spool.tile([128, 512], F32, tag="recip_bc", bufs=1)
                    nc.gpsimd.partition_broadcast(rb, rc)
                    nc.vector.tensor_tensor(ctx_sb[:, h, :], ctxr, rb,
                                            mybir.AluOpType.mult)

                # ---- output projection for this seq chunk ----
                for hidc in range(HID // 128):
                    owt = wpool.tile([D_V, HPC, 128], mm_dt, tag="w_o")
                    nc.sync.dma_start(
                        out=owt, in_=ow.ap()[:, :, hidc * 128:(hidc + 1) * 128])
                    acc = ppool.tile([128, 512], F32, tag="p_a", name="acc_o")
                    for h in range(HPC):
                        nc.tensor.matmul(acc, owt[:, h, :], ctx_sb[:, h, :],
                                         start=(h == 0), stop=(h == HPC - 1))
                    o = spool.tile([128, 512], mm_dt, tag="oout", bufs=1)
                    nc.vector.tensor_copy(o, acc)
                    nc.gpsimd.dma_start(
                        out=out_t.ap()[hidc * 128:(hidc + 1) * 128,
                                       sqc * 512:(sqc + 1) * 512],
                        in_=o)

    nc.compile()
    return nc


# ------------------------------------------------------------- host side --
def _rope_tables():
    inv_freq = 1.0 / (ROPE_THETA ** (np.arange(0, D_ROPE, 2, dtype=np.float64) / D_ROPE))
    t = np.arange(S, dtype=np.float64)
    freqs = np.outer(t, inv_freq)                    # [S, 32]
    emb = np.concatenate([freqs, freqs], axis=-1)    # [S, 64]
    return (np.cos(emb).astype(np.float32).T.copy(),
            np.sin(emb).astype(np.float32).T.copy())  # [64, S]


_E_PERM = np.concatenate([np.arange(0, D_ROPE, 2), np.arange(1, D_ROPE, 2)])


def _rope_expand(Wpe):
    """[n, 64] rope weight cols -> [n, 128]: [even/odd-reordered | rot-half signed]."""
    Y = Wpe[:, _E_PERM]
    R = np.concatenate([-Y[:, D_ROPE // 2:], Y[:, :D_ROPE // 2]], axis=1)
    return np.concatenate([Y, R], axis=1)


def _chunk_rows(a, p=128):
    """[R, ...] -> [p, R//p, ...] grouping rows into chunks of p."""
    R, Cs = a.shape[0], a.shape[1:]
    return np.ascontiguousarray(a.reshape(R // p, p, *Cs).transpose(
        1, 0, *range(2, a.ndim + 1)))


def _prep_inputs(hidden_states, w_dq, q_a_ln_w, w_uq, kv_a_w, kv_a_ln_w, kv_b_w, o_w):
    bf = ml_dtypes.bfloat16
    s_loc = S // 4
    cosT, sinT = _rope_tables()

    wuq_eff = (np.asarray(q_a_ln_w)[:, None] * np.asarray(w_uq)).reshape(Q_LORA, H, D_Q)
    head_blocks = []
    for h in range(H):
        head_blocks.append(np.concatenate(
            [wuq_eff[:, h, :D_NOPE], _rope_expand(wuq_eff[:, h, D_NOPE:])], axis=1))
    wuq_x = np.stack(head_blocks, axis=1)            # [1536, 16, 256]

    kv_a = np.asarray(kv_a_w)
    wkva_x = np.concatenate([kv_a[:, :KV_LORA], _rope_expand(kv_a[:, KV_LORA:])],
                            axis=1).astype(bf)       # [2048, 640]
    wkva_p = _chunk_rows(wkva_x)                     # [128, 16, 640]
    wkvb_eff = (np.asarray(kv_a_ln_w)[:, None] * np.asarray(kv_b_w)).reshape(KV_LORA, H, 256)
    ow_r = np.asarray(o_w).reshape(H, D_V, HID)

    c_idx = np.arange(512)[None, :]
    r_idx = np.arange(128)[:, None]
    masks = np.stack([(c_idx >= 128 * dd + r_idx) for dd in range(4)],
                     axis=1).astype(bf)              # [128, 4, 512]

    wdq_b = np.asarray(w_dq).astype(bf)
    hs = np.asarray(hidden_states)

    in_maps = []
    for c in range(N_CORES):
        b, hg = c // 4, c % 4
        s0 = 512 * hg
        xt_full = np.ascontiguousarray(hs[b].T).astype(bf)
        wuq_c = wuq_x[:, HPC * hg: HPC * (hg + 1), :].reshape(
            Q_LORA, HPC * 256).astype(bf)
        wkvb_c = wkvb_eff[:, HPC * hg: HPC * (hg + 1), :].astype(bf)
        in_maps.append({
            "xt": xt_full,
            "xt_loc": np.ascontiguousarray(xt_full[:, s0:s0 + s_loc]),
            "wdq": wdq_b,
            "wuq": _chunk_rows(wuq_c),               # [128, 12, 1024]
            "wkva": wkva_p,
            "wkvb": _chunk_rows(wkvb_c),             # [128, 4, 4, 256]
            "ow": np.ascontiguousarray(
                ow_r[HPC * hg: HPC * (hg + 1)].transpose(1, 0, 2)).astype(bf),
            "cos_f": cosT.astype(bf),
            "sin_f": sinT.astype(bf),
            "masks": masks,
        })
    return in_maps


def _postprocess(results):
    out = np.empty((B, S, HID), dtype=np.float32)
    for b in range(B):
        acc = results[4 * b]["out_t"].astype(np.float32)
        for c in GROUPS[b][1:]:
            acc = acc + results[c]["out_t"].astype(np.float32)
        out[b] = acc.T
    return out


def kernel(**inputs):
    key = (str(MM_DT),)
    if key not in _CACHE:
        _CACHE[key] = build_kernel(MM_DT)
    nc = _CACHE[key]
    in_maps = _prep_inputs(**inputs)
    r = run_bass_kernel_spmd(nc, in_maps, core_ids=list(range(N_CORES)))
    return _postprocess(r.results)
